# revision 14
# baseline (speedup 1.0000x reference)
"""Trainium2 Bass kernel for nn_Angles2Backbone.

Full inputs:  input [1024, 3, 512] f32 (phi/psi/omega dihedrals), angles_length [1024] i64.
Full output:  [1024, 4608] f32 backbone coords (N, CA, C per residue, xyz interleaved).

Strategy: pure data parallelism — 128 protein chains per NeuronCore (batch on the
partition axis), 512 residues on the free axis.

Layout: residues are stored POSITION-MAJOR ("permuted"): residue r = 8j + i lives
at column i*64 + j.  Every step of the blocked quaternion scan is then a
contiguous [128, 64] op; block-prefix rotation uses combo planes replicated by
cheap doubling copies so all hot ops stay contiguous (strided/broadcast APs
measured 2-4x slower on DVE).  The layout is undone only in the final
interleaved coordinate writes (strided writes are cheap on DVE).

Pipeline / engine split:
  A  permute + trig                      (ScalarE)
  B1 residue rotor Q_r, f32              (DVE + some Pool; feeds the scan)
  C  blocked quat scan: 7 serial in-block combines + 6 doubling combines
     over 64 block aggregates, f32       (DVE comps w/x/y, Pool comp z)
  B2 intra-residue offsets u0/u1/u2 bf16 (Pool + ScalarE, fills scan gaps)
  D  two-stage rotation: w' = R(L_ex) u then w = R(P_ex) w', bf16 (DVE-lean)
  E  hierarchical cumsum of w2 (f32) + coords
"""

import math

import numpy as np

N_CORES = 8
B_FULL = 1024
L = 512
CB = B_FULL // N_CORES  # 128 chains per core
NB = 128  # scan blocks
G = L // NB  # positions per block
NL = L - NB  # 448

R_CA_C = 1.525
R_C_N = 1.330
R_N_CA = 1.460
CA_C_N = math.pi - 2.1186
C_N_CA = math.pi - 1.9391
N_CA_C = math.pi - 2.061

B_K = [C_N_CA, N_CA_C, CA_C_N]
R_KC = [R_C_N, R_N_CA, R_CA_C]

HALF_PI = math.pi / 2.0

_QPAIRS = [
    (0, 0), (1, 1), (2, 2), (3, 3),  # w
    (0, 1), (1, 0), (2, 3), (3, 2),  # x
    (0, 2), (1, 3), (2, 0), (3, 1),  # y
    (0, 3), (1, 2), (2, 1), (3, 0),  # z
]

_COMBO_NAMES = ("S1", "S2", "S3", "A1", "D1", "A2", "D2", "A3", "D3")


def _body(ctx, tc, out_ap, inp_ap, lens_ap):
    import concourse.mybir as mybir

    nc = tc.nc
    f32 = mybir.dt.float32
    bf16 = mybir.dt.bfloat16
    Alu = mybir.AluOpType
    Act = mybir.ActivationFunctionType

    cb0h, sb0h = math.cos(B_K[0] / 2), math.sin(B_K[0] / 2)
    cb1h, sb1h = math.cos(B_K[1] / 2), math.sin(B_K[1] / 2)
    cb2h, sb2h = math.cos(B_K[2] / 2), math.sin(B_K[2] / 2)
    cb0f, sb0f = math.cos(B_K[0]), math.sin(B_K[0])
    cb1f, sb1f = math.cos(B_K[1]), math.sin(B_K[1])

    def ttv(o, a, b, op):
        nc.vector.tensor_tensor(out=o, in0=a, in1=b, op=op)

    def ttp(o, a, b, op):
        nc.gpsimd.tensor_tensor(out=o, in0=a, in1=b, op=op)

    def stt(o, in0, scalar, in1, op0, op1):
        nc.vector.scalar_tensor_tensor(out=o, in0=in0, scalar=scalar, in1=in1,
                                       op0=op0, op1=op1)

    def ts(o, a, s1, s2=None):
        nc.scalar.activation(o, a, Act.Identity,
                             bias=(0.0 if s2 is None else cval(s2)), scale=s1)

    def ts_v(o, a, s1):
        nc.vector.tensor_scalar(out=o, in0=a, scalar1=s1, scalar2=None,
                                op0=Alu.mult)

    def acopy(o, a):
        nc.scalar.activation(o, a, Act.Copy, bias=0.0, scale=1.0)

    # ------------------------------------------------------------------ pools
    persist = ctx.enter_context(tc.tile_pool(name="persist", bufs=1))
    Qp = [persist.tile([CB, L], f32, name=f"Qp_{c}") for c in range(4)]
    u0 = [persist.tile([CB, L], bf16, name=f"u0_{d}") for d in range(2)]
    u1 = [persist.tile([CB, L], bf16, name=f"u1_{d}") for d in range(3)]
    u2 = [persist.tile([CB, L], bf16, name=f"u2_{d}") for d in range(3)]
    wp0 = [persist.tile([CB, L], bf16, name=f"wp0_{d}") for d in range(3)]
    wp1 = [persist.tile([CB, L], bf16, name=f"wp1_{d}") for d in range(3)]
    wp2 = [persist.tile([CB, L], bf16, name=f"wp2_{d}") for d in range(3)]
    w0 = [persist.tile([CB, L], bf16, name=f"w0_{d}") for d in range(3)]
    w1 = [persist.tile([CB, L], bf16, name=f"w1_{d}") for d in range(3)]
    w2 = [persist.tile([CB, L], f32, name=f"w2_{d}") for d in range(3)]
    cfb = [persist.tile([CB, L], bf16, name=f"cfb{i}") for i in range(3)]
    sfb = [persist.tile([CB, L], bf16, name=f"sfb{i}") for i in range(3)]
    out_sb = persist.tile([CB, 9 * L], f32, name="out_sb")
    ones = persist.tile([CB, NB], f32, name="ones")
    mask = persist.tile([CB, L], f32, name="mask")
    lens_sb = persist.tile([CB, 1], f32, name="lens_sb")

    nc.gpsimd.memset(ones[:], 1.0)
    nc.sync.dma_start(lens_sb[:], lens_ap)

    _consts = {}

    def cval(v):
        if v not in _consts:
            t = persist.tile([CB, 1], f32, name=f"cval_{len(_consts)}")
            nc.gpsimd.memset(t[:], v)
            _consts[v] = t[:]
        return _consts[v]

    # ---------------- Phase A: load + permute + trig --------------------------
    phase_b = tc.tile_pool(name="phase_b", bufs=1)
    pb = phase_b.__enter__()
    dih = pb.tile([CB, 3, L], f32, name="dih")
    for k in range(3):
        nc.sync.dma_start(dih[:][:, k, :], inp_ap[:, k, :])

    def bplane(name, dt_=f32):
        return pb.tile([CB, L], dt_, name=name)

    # permuted angle planes: pang[k][col i*64+j] = dih[k][col 8j+i]
    pang = [bplane(f"pang{k}") for k in range(3)]
    for k, eng in enumerate((nc.vector, nc.gpsimd, nc.scalar)):
        psrc = dih[:][:, k, :].rearrange("p (j i) -> p i j", i=G)
        pdst = pang[k][:].rearrange("p (i j) -> p i j", j=NB)
        if eng is nc.scalar:
            acopy(pdst, psrc)
        else:
            eng.tensor_copy(pdst, psrc)
    phi, psi, omg = pang[0][:], pang[1][:], pang[2][:]

    # ScalarE Sin domain is [-pi, pi]; cosines via cos(y) = 1 - 2 sin^2(y/2)
    cf = [bplane(f"cf{i}") for i in range(3)]
    sf = [bplane(f"sf{i}") for i in range(3)]
    sq = bplane("sqtmp")
    sOh = bplane("sOh")
    for i, ang in enumerate((phi, psi, omg)):
        nc.scalar.activation(sf[i][:], ang, Act.Sin, bias=0.0, scale=1.0)
        half = sOh if i == 2 else sq
        nc.scalar.activation(half[:], ang, Act.Sin, bias=0.0, scale=0.5)
        ttv(cf[i][:], half[:], half[:], Alu.mult)
        nc.vector.tensor_scalar(out=cf[i][:], in0=cf[i][:], scalar1=-2.0,
                                scalar2=1.0, op0=Alu.mult, op1=Alu.add)
        nc.vector.tensor_copy(cfb[i][:], cf[i][:])
        nc.gpsimd.tensor_copy(sfb[i][:], sf[i][:])

    ssum = bplane("ssum")
    sdif = bplane("sdif")
    ttv(ssum[:], phi, psi, Alu.add)
    ttv(sdif[:], phi, psi, Alu.subtract)

    cS = bplane("cS"); sS = bplane("sS")
    cD = bplane("cD"); sD = bplane("sD")
    cOh = bplane("cOh")
    nc.scalar.activation(sS[:], ssum[:], Act.Sin, bias=0.0, scale=0.5)
    nc.scalar.activation(sD[:], sdif[:], Act.Sin, bias=0.0, scale=0.5)
    nc.scalar.activation(cS[:], ssum[:], Act.Sin, bias=0.0, scale=0.25)
    ttv(cS[:], cS[:], cS[:], Alu.mult)
    nc.vector.tensor_scalar(out=cS[:], in0=cS[:], scalar1=-2.0, scalar2=1.0,
                            op0=Alu.mult, op1=Alu.add)
    nc.scalar.activation(cD[:], sdif[:], Act.Sin, bias=0.0, scale=0.25)
    ttv(cD[:], cD[:], cD[:], Alu.mult)
    nc.vector.tensor_scalar(out=cD[:], in0=cD[:], scalar1=-2.0, scalar2=1.0,
                            op0=Alu.mult, op1=Alu.add)
    nc.scalar.activation(cOh[:], omg, Act.Sin, bias=cval(HALF_PI), scale=0.5)

    # mask = (r < length); iota value r = 8j+i at permuted col i*64+j
    iota = bplane("iota")
    nc.gpsimd.iota(iota[:], pattern=[[1, G], [G, NB]], base=0,
                   channel_multiplier=0, allow_small_or_imprecise_dtypes=True)
    nc.vector.tensor_scalar(out=mask[:], in0=iota[:], scalar1=lens_sb[:],
                            scalar2=None, op0=Alu.is_lt)

    # ---------------- Phase B1: residue rotor Q (f32) -------------------------
    q2 = [bplane(f"q2_{c}") for c in range(4)]
    ts_v(q2[0][:], cS[:], cb0h)
    ts_v(q2[1][:], cD[:], sb0h)
    ts_v(q2[2][:], sD[:], sb0h)
    ts_v(q2[3][:], sS[:], cb0h)

    q3 = [bplane(f"q3_{c}") for c in range(4)]
    qt = [bplane(f"qt_{c}") for c in range(4)]
    ts_v(qt[0][:], q2[1][:], sb1h)
    stt(q3[0][:], q2[0][:], cb1h, qt[0][:], Alu.mult, Alu.subtract)
    ts_v(qt[1][:], q2[0][:], sb1h)
    stt(q3[1][:], q2[1][:], cb1h, qt[1][:], Alu.mult, Alu.add)
    ts_v(qt[2][:], q2[3][:], sb1h)
    stt(q3[2][:], q2[2][:], cb1h, qt[2][:], Alu.mult, Alu.add)
    ts_v(qt[3][:], q2[2][:], sb1h)
    stt(q3[3][:], q2[3][:], cb1h, qt[3][:], Alu.mult, Alu.subtract)

    # q4 = q3 * qz(omega/2); reuse q2 tiles for q4, qt for partial products
    q4 = q2
    for c, (src, shuf, op) in enumerate((
            (q3[0], q3[3], Alu.subtract), (q3[1], q3[2], Alu.add),
            (q3[2], q3[1], Alu.subtract), (q3[3], q3[0], Alu.add))):
        e1, e2 = (ttv, ttp) if c % 2 else (ttp, ttv)
        e1(q4[c][:], src[:], cOh[:], Alu.mult)
        e2(qt[c][:], shuf[:], sOh[:], Alu.mult)
        ttv(q4[c][:], q4[c][:], qt[c][:], op)

    ts_v(qt[0][:], q4[1][:], sb2h)
    stt(Qp[0][:], q4[0][:], cb2h, qt[0][:], Alu.mult, Alu.subtract)
    ts_v(qt[1][:], q4[0][:], sb2h)
    stt(Qp[1][:], q4[1][:], cb2h, qt[1][:], Alu.mult, Alu.add)
    ts_v(qt[2][:], q4[3][:], sb2h)
    stt(Qp[2][:], q4[2][:], cb2h, qt[2][:], Alu.mult, Alu.add)
    ts_v(qt[3][:], q4[2][:], sb2h)
    stt(Qp[3][:], q4[3][:], cb2h, qt[3][:], Alu.mult, Alu.subtract)

    phase_b.__exit__(None, None, None)

    # ---------------- Phase C: blocked quaternion scan (f32, contiguous) ------
    scan_pool = ctx.enter_context(tc.tile_pool(name="scan", bufs=1))
    tmp = [scan_pool.tile([CB, NB], f32, name=f"tmp_{i}") for i in range(16)]

    def qcombine(Lap, Rap, Oap, n):
        """O = L x R; comps w,x,y on DVE, comp z on Pool."""
        eng = [ttv, ttv, ttv, ttp]
        mv = []
        for k, (a, b) in enumerate(_QPAIRS):
            dst = tmp[k][:][:, 0:n]
            eng[k // 4](dst, Lap[a], Rap[b], Alu.mult)
            mv.append(dst)
        specs = [
            (0, 0, 1, Alu.subtract, 2, 3, Alu.add, Alu.subtract),
            (1, 4, 5, Alu.add, 6, 7, Alu.subtract, Alu.add),
            (2, 8, 9, Alu.subtract, 10, 11, Alu.add, Alu.add),
            (3, 12, 13, Alu.add, 15, 14, Alu.subtract, Alu.add),
        ]
        for comp, a, b, opab, c_, d_, opcd, opf in specs:
            e = eng[comp]
            e(mv[a], mv[a], mv[b], opab)
            e(mv[c_], mv[c_], mv[d_], opcd)
            e(Oap[comp], mv[a], mv[c_], opf)

    for i in range(1, G):
        Lap = [Qp[c][:][:, (i - 1) * NB:i * NB] for c in range(4)]
        Rap = [Qp[c][:][:, i * NB:(i + 1) * NB] for c in range(4)]
        qcombine(Lap, Rap, Rap, NB)

    s = 1
    while s < NB:
        base = (G - 1) * NB
        Lap = [Qp[c][:][:, base:base + NB - s] for c in range(4)]
        Rap = [Qp[c][:][:, base + s:base + NB] for c in range(4)]
        qcombine(Lap, Rap, Rap, NB - s)
        s *= 2

    # ---------------- Phase B2: u vectors (bf16; Pool + ScalarE) --------------
    # issued after the scan so these fill Pool/ACT gaps without delaying DVE
    p1 = scan_pool.tile([CB, L], bf16, name="p1")
    p2 = scan_pool.tile([CB, L], bf16, name="p2")
    p3 = scan_pool.tile([CB, L], bf16, name="p3")
    p4 = scan_pool.tile([CB, L], bf16, name="p4")
    ttp(p1[:], cfb[0][:], cfb[1][:], Alu.mult)
    ttp(p2[:], sfb[0][:], sfb[1][:], Alu.mult)
    ttp(p3[:], sfb[0][:], cfb[1][:], Alu.mult)
    ttp(p4[:], cfb[0][:], sfb[1][:], Alu.mult)

    v0 = [scan_pool.tile([CB, L], bf16, name=f"v0_{d}") for d in range(3)]
    bt1 = scan_pool.tile([CB, L], bf16, name="bt1")
    bt2 = scan_pool.tile([CB, L], bf16, name="bt2")
    ts(bt1[:], p2[:], -cb0f)
    ttp(v0[0][:], bt1[:], p1[:], Alu.add)
    ts(bt2[:], p4[:], cb0f)
    ttp(v0[1][:], bt2[:], p3[:], Alu.add)
    ts(v0[2][:], sfb[1][:], sb0f)

    ts(u0[0][:], cfb[0][:], R_KC[0])
    ts(u0[1][:], sfb[0][:], R_KC[0])
    nc.gpsimd.memset(u0[0][:][:, 0:1], 0.0)
    nc.gpsimd.memset(u0[1][:][:, 0:1], 0.0)

    ts(bt1[:], v0[0][:], R_KC[1])
    ttp(u1[0][:], bt1[:], u0[0][:], Alu.add)
    ts(bt2[:], v0[1][:], R_KC[1])
    ttp(u1[1][:], bt2[:], u0[1][:], Alu.add)
    ts(u1[2][:], v0[2][:], R_KC[1])

    c1x = scan_pool.tile([CB, L], bf16, name="c1x")
    c1y = scan_pool.tile([CB, L], bf16, name="c1y")
    c1z = scan_pool.tile([CB, L], bf16, name="c1z")
    ts(c1x[:], sfb[0][:], sb0f * sb1f)
    ts(bt1[:], p3[:], -cb0f * cb1f)
    ttp(c1x[:], bt1[:], c1x[:], Alu.add)
    ts(bt2[:], p4[:], -cb1f)
    ttp(c1x[:], bt2[:], c1x[:], Alu.add)
    ts(c1y[:], cfb[0][:], -sb0f * sb1f)
    ts(bt1[:], p1[:], cb0f * cb1f)
    ttp(c1y[:], bt1[:], c1y[:], Alu.add)
    ts(bt2[:], p2[:], -cb1f)
    ttp(c1y[:], bt2[:], c1y[:], Alu.add)
    ts(c1z[:], cfb[1][:], sb0f * cb1f, cb0f * sb1f)

    for d, c1 in enumerate((c1x, c1y, c1z)):
        qa = scan_pool.tile([CB, L], bf16, name=f"u2t_{d}")
        qb = scan_pool.tile([CB, L], bf16, name=f"u2s_{d}")
        ttp(qa[:], cfb[2][:], v0[d][:], Alu.mult)
        ttp(qb[:], sfb[2][:], c1[:], Alu.mult)
        ttp(qa[:], qa[:], qb[:], Alu.add)
        ts(qb[:], qa[:], R_KC[2])
        ttp(u2[d][:], qb[:], u1[d][:], Alu.add)

    # ---------------- Phase D: two-stage rotation -----------------------------
    rot_pool = ctx.enter_context(tc.tile_pool(name="rot", bufs=1))

    # stage 1: w' = R(L_ex) u  (local exclusive prefix = contiguous shift)
    Qlb = [rot_pool.tile([CB, NL], bf16, name=f"Qlb_{c}") for c in range(4)]
    for c in range(4):
        acopy(Qlb[c][:], Qp[c][:][:, 0:NL])
    lw, lx, ly, lz = [Qlb[c][:] for c in range(4)]

    prod_tmp = {}
    for nm in ("xx", "yy", "zz", "xy", "xz", "yz", "wx", "wy", "wz"):
        prod_tmp[nm] = rot_pool.tile([CB, NL], bf16, name=f"l1p_{nm}")
    L1c = {}
    for nm in _COMBO_NAMES:
        L1c[nm] = rot_pool.tile([CB, NL], bf16, name=f"l1c_{nm}")
    pr = {k: prod_tmp[k][:] for k in prod_tmp}
    ttv(pr["xx"], lx, lx, Alu.mult)
    ttv(pr["yy"], ly, ly, Alu.mult)
    ttv(pr["zz"], lz, lz, Alu.mult)
    ttv(pr["xy"], lx, ly, Alu.mult)
    ttp(pr["xz"], lx, lz, Alu.mult)
    ttp(pr["yz"], ly, lz, Alu.mult)
    ttv(pr["wx"], lw, lx, Alu.mult)
    ttv(pr["wy"], lw, ly, Alu.mult)
    ttv(pr["wz"], lw, lz, Alu.mult)
    ttv(L1c["S1"][:], pr["yy"], pr["zz"], Alu.add)
    ttv(L1c["S2"][:], pr["xx"], pr["zz"], Alu.add)
    ttv(L1c["S3"][:], pr["xx"], pr["yy"], Alu.add)
    ttv(L1c["A1"][:], pr["xy"], pr["wz"], Alu.add)
    ttv(L1c["D1"][:], pr["xy"], pr["wz"], Alu.subtract)
    ttp(L1c["A2"][:], pr["xz"], pr["wy"], Alu.add)
    ttp(L1c["D2"][:], pr["xz"], pr["wy"], Alu.subtract)
    ttv(L1c["A3"][:], pr["yz"], pr["wx"], Alu.add)
    ttv(L1c["D3"][:], pr["yz"], pr["wx"], Alu.subtract)

    rta = rot_pool.tile([CB, L], bf16, name="rta")
    rtb = rot_pool.tile([CB, L], bf16, name="rtb")
    rtc = rot_pool.tile([CB, L], bf16, name="rtc")

    def rot_core(C, vx, vy, vz, ta, tb, tc_, outs, final):
        """outs = R @ (vx,vy,vz); vz may be None (zero).  final(out, t, base)
        emits 'out = base + 2*t'."""
        ttv(ta, vy, C("D1"), Alu.mult)
        if vz is not None:
            ttp(tb, vz, C("A2"), Alu.mult)
            ttv(ta, ta, tb, Alu.add)
        ttv(tb, vx, C("S1"), Alu.mult)
        ttv(ta, ta, tb, Alu.subtract)
        final(outs[0], ta, vx)
        ttv(tb, vx, C("A1"), Alu.mult)
        if vz is not None:
            ttp(tc_, vz, C("D3"), Alu.mult)
            ttv(tb, tb, tc_, Alu.add)
        ttv(tc_, vy, C("S2"), Alu.mult)
        ttv(tb, tb, tc_, Alu.subtract)
        final(outs[1], tb, vy)
        ttv(tc_, vx, C("D2"), Alu.mult)
        ttp(ta, vy, C("A3"), Alu.mult)
        ttv(tc_, tc_, ta, Alu.add)
        if vz is not None:
            ttv(ta, vz, C("S3"), Alu.mult)
            ttv(tc_, tc_, ta, Alu.subtract)
            final(outs[2], tc_, vz)
        else:
            final(outs[2], tc_, None)

    def final_stt(out_ap, t_ap, base_ap):
        if base_ap is None:
            ts_v(out_ap, t_ap, 2.0)
        else:
            stt(out_ap, t_ap, 2.0, base_ap, Alu.mult, Alu.add)

    def hi(p):  # cols [64:512]
        return p[:][:, NB:L]

    def lo448(p):  # cols [0:448]
        return p[:][:, 0:NL]

    def rotate1(uvec, wvec):
        C = lambda nm: L1c[nm][:]
        vz = hi(uvec[2]) if uvec[2] is not None else None
        rot_core(C, hi(uvec[0]), hi(uvec[1]), vz,
                 lo448(rta), lo448(rtb), lo448(rtc),
                 [hi(wvec[0]), hi(wvec[1]), hi(wvec[2])], final_stt)

    rotate1((u0[0], u0[1], None), wp0)
    rotate1((u1[0], u1[1], u1[2]), wp1)
    rotate1((u2[0], u2[1], u2[2]), wp2)

    # identity part: position i=0 (cols [0:64]) gets w' = u
    for src, dst in ((u0[0], wp0[0]), (u0[1], wp0[1]),
                     (u1[0], wp1[0]), (u1[1], wp1[1]), (u1[2], wp1[2]),
                     (u2[0], wp2[0]), (u2[1], wp2[1]), (u2[2], wp2[2])):
        acopy(dst[:][:, 0:NB], src[:][:, 0:NB])
    nc.gpsimd.memset(wp0[2][:][:, 0:NB], 0.0)

    # stage 2: w = R(P_ex) w'.  Combos of the exclusive block prefix live on
    # [CB,64] (col j <- aggregate j-1; col 0 = 0 = identity rotation) and are
    # replicated to [CB,512] with doubling copies so rotation ops stay
    # contiguous.
    aggw = [Qp[c][:][:, (G - 1) * NB:(G - 1) * NB + NB - 1] for c in range(4)]
    p2p = {}
    for nm in ("xx", "yy", "zz", "xy", "xz", "yz", "wx", "wy", "wz"):
        p2p[nm] = rot_pool.tile([CB, NB], bf16, name=f"p2p_{nm}")
    P2r = {}
    for nm in _COMBO_NAMES:
        P2r[nm] = rot_pool.tile([CB, L], bf16, name=f"p2r_{nm}")
        nc.vector.memset(P2r[nm][:][:, 0:1], 0.0)

    def pp(nm):
        return p2p[nm][:][:, 1:NB]

    def p2c(nm):
        return P2r[nm][:][:, 1:NB]

    pw_, px_, py_, pz_ = aggw
    ttv(pp("xx"), px_, px_, Alu.mult)
    ttv(pp("yy"), py_, py_, Alu.mult)
    ttv(pp("zz"), pz_, pz_, Alu.mult)
    ttv(pp("xy"), px_, py_, Alu.mult)
    ttv(pp("xz"), px_, pz_, Alu.mult)
    ttv(pp("yz"), py_, pz_, Alu.mult)
    ttv(pp("wx"), pw_, px_, Alu.mult)
    ttv(pp("wy"), pw_, py_, Alu.mult)
    ttv(pp("wz"), pw_, pz_, Alu.mult)
    ttv(p2c("S1"), pp("yy"), pp("zz"), Alu.add)
    ttv(p2c("S2"), pp("xx"), pp("zz"), Alu.add)
    ttv(p2c("S3"), pp("xx"), pp("yy"), Alu.add)
    ttv(p2c("A1"), pp("xy"), pp("wz"), Alu.add)
    ttv(p2c("D1"), pp("xy"), pp("wz"), Alu.subtract)
    ttv(p2c("A2"), pp("xz"), pp("wy"), Alu.add)
    ttv(p2c("D2"), pp("xz"), pp("wy"), Alu.subtract)
    ttv(p2c("A3"), pp("yz"), pp("wx"), Alu.add)
    ttv(p2c("D3"), pp("yz"), pp("wx"), Alu.subtract)

    # replicate the [0:NB] block across the full plane (doubling, ScalarE)
    for nm in _COMBO_NAMES:
        pl = P2r[nm][:]
        seg = NB
        while seg < L:
            acopy(pl[:, seg:min(2 * seg, L)], pl[:, 0:min(seg, L - seg)])
            seg *= 2

    def rotate2(wsrc, wdst):
        C = lambda nm: P2r[nm][:]
        rot_core(C, wsrc[0][:], wsrc[1][:], wsrc[2][:],
                 rta[:], rtb[:], rtc[:],
                 [wdst[0][:], wdst[1][:], wdst[2][:]], final_stt)

    rotate2(wp0, w0)
    rotate2(wp1, w1)
    rotate2(wp2, w2)

    # ---------------- Phase E: hierarchical cumsum + coords -------------------
    for i in range(1, G):
        for d in range(3):
            e = ttv if (i + d) % 4 else ttp
            e(w2[d][:][:, i * NB:(i + 1) * NB],
              w2[d][:][:, (i - 1) * NB:i * NB],
              w2[d][:][:, i * NB:(i + 1) * NB], Alu.add)

    Ot = [rot_pool.tile([CB, NB + 1], f32, name=f"Ot_{d}") for d in range(3)]
    Orr = [rot_pool.tile([CB, L], f32, name=f"Or_{d}") for d in range(3)]
    for d in range(3):
        nc.vector.memset(Ot[d][:][:, 0:1], 0.0)
        nc.vector.tensor_tensor_scan(
            out=Ot[d][:][:, 1:NB + 1],
            data0=ones[:],
            data1=w2[d][:][:, (G - 1) * NB:G * NB],
            initial=0.0, op0=Alu.mult, op1=Alu.add,
        )
        pl = Orr[d][:]
        acopy(pl[:, 0:NB], Ot[d][:][:, 0:NB])
        seg = NB
        while seg < L:
            acopy(pl[:, seg:min(2 * seg, L)], pl[:, 0:min(seg, L - seg)])
            seg *= 2

    Binc = [rot_pool.tile([CB, L], f32, name=f"Binc_{d}") for d in range(3)]
    Bex = [rot_pool.tile([CB, L], f32, name=f"Bex_{d}") for d in range(3)]
    for d in range(3):
        e = ttv if d != 1 else ttp
        e(Binc[d][:], w2[d][:], Orr[d][:], Alu.add)
        e(Bex[d][:][:, NB:L], w2[d][:][:, 0:NL], Orr[d][:][:, 0:NL], Alu.add)
        acopy(Bex[d][:][:, 0:NB], Ot[d][:][:, 0:NB])

    # coords: out column 9r + 3k + d, r = 8j+i, read from permuted col i*64+j
    def outview(k, d):
        return out_sb[:].rearrange("p (j i q) -> p i j q", j=NB, i=G,
                                   q=9)[:, :, :, 3 * k + d]

    def pview(plane):
        return plane[:].rearrange("p (i j) -> p i j", j=NB)

    maskv = mask[:].rearrange("p (i j) -> p i j", j=NB)
    ct = [rot_pool.tile([CB, L], f32, name=f"ct_{d}") for d in range(3)]
    for k, wk in enumerate((w0, w1, None)):
        for d in range(3):
            if k == 2:
                ttv(outview(2, d), pview(Binc[d]), maskv, Alu.mult)
            else:
                e = ttv if (k + d) % 3 else ttp
                e(ct[d][:], wk[d][:], Bex[d][:], Alu.add)
                ttv(outview(k, d), pview(ct[d]), maskv, Alu.mult)

    nc.sync.dma_start(out_ap, out_sb[:])


_CACHE = {}


def _build():
    from contextlib import ExitStack

    import concourse.bacc as bacc
    import concourse.mybir as mybir
    import concourse.tile as tile

    nc = bacc.Bacc("TRN2", target_bir_lowering=False, debug=False,
                   num_devices=N_CORES)
    inp = nc.dram_tensor("input", [CB, 3, L], mybir.dt.float32,
                         kind="ExternalInput").ap()
    lens = nc.dram_tensor("lens", [CB, 1], mybir.dt.float32,
                          kind="ExternalInput").ap()
    out = nc.dram_tensor("out", [CB, 9 * L], mybir.dt.float32,
                         kind="ExternalOutput").ap()
    with tile.TileContext(nc) as tc_ctx, ExitStack() as ctx:
        _body(ctx, tc_ctx, out, inp, lens)
    nc.compile()
    return nc


def get_nc():
    if "nc" not in _CACHE:
        _CACHE["nc"] = _build()
    return _CACHE["nc"]


def make_in_maps(input, angles_length):
    inp = np.ascontiguousarray(np.asarray(input, dtype=np.float32))
    lens = np.asarray(angles_length).astype(np.float32).reshape(B_FULL, 1)
    in_maps = []
    for i in range(N_CORES):
        sl = slice(i * CB, (i + 1) * CB)
        in_maps.append({
            "input": np.ascontiguousarray(inp[sl]),
            "lens": np.ascontiguousarray(lens[sl]),
        })
    return in_maps


def kernel(input, angles_length):
    from concourse.bass_utils import run_bass_kernel_spmd

    nc = get_nc()
    in_maps = make_in_maps(input, angles_length)
    res = run_bass_kernel_spmd(nc, in_maps, core_ids=list(range(N_CORES)))
    outs = [res.results[i]["out"] for i in range(N_CORES)]
    return np.concatenate(outs, axis=0).astype(np.float32)


# revision 16
# speedup vs baseline: 1.0251x; 1.0251x over previous
"""Trainium2 Bass kernel for nn_Angles2Backbone.

Full inputs:  input [1024, 3, 512] f32 (phi/psi/omega dihedrals), angles_length [1024] i64.
Full output:  [1024, 4608] f32 backbone coords (N, CA, C per residue, xyz interleaved).

Strategy: pure data parallelism — 128 protein chains per NeuronCore (batch on
partitions), 512 residues on the free axis.

Layout: residues are POSITION-MAJOR ("permuted"): residue r = G*j + i lives at
column i*NB + j (G=4 positions, NB=128 blocks).  Every scan step is then a
contiguous [128, NB] op and the block-prefix application is one contiguous
combine against a replicated prefix plane.

Pipeline (slab-pipelined start):
  per slab i: trig (ScalarE Sin) + residue rotor Q (DVE, f32) + serial scan
  step i (combines slab i-1 into slab i) — DVE never waits on a long serial
  ScalarE prefix.  Then: doubling scan over the 128 block aggregates (f32);
  intra-residue offsets u0/u1/u2 in bf16 on Pool/ScalarE (fills scan gaps);
  one contiguous prefix-apply combine; single-stage rotation by the exclusive
  cumulative rotor (bf16, errors are local); hierarchical cumsum (f32) and
  coordinate assembly, un-permuted by ScalarE interleave copies.
"""

import math

import numpy as np

N_CORES = 8
B_FULL = 1024
L = 512
CB = B_FULL // N_CORES  # 128 chains per core
NB = 128  # scan blocks
G = L // NB  # 4 positions per block
NL = L - NB  # 384

R_CA_C = 1.525
R_C_N = 1.330
R_N_CA = 1.460
CA_C_N = math.pi - 2.1186
C_N_CA = math.pi - 1.9391
N_CA_C = math.pi - 2.061

B_K = [C_N_CA, N_CA_C, CA_C_N]
R_KC = [R_C_N, R_N_CA, R_CA_C]

HALF_PI = math.pi / 2.0

_QPAIRS = [
    (0, 0), (1, 1), (2, 2), (3, 3),  # w
    (0, 1), (1, 0), (2, 3), (3, 2),  # x
    (0, 2), (1, 3), (2, 0), (3, 1),  # y
    (0, 3), (1, 2), (2, 1), (3, 0),  # z
]

_COMBO_NAMES = ("S1", "S2", "S3", "A1", "D1", "A2", "D2", "A3", "D3")
_PROD_NAMES = ("xx", "yy", "zz", "xy", "xz", "yz", "wx", "wy", "wz")


def _body(ctx, tc, out_ap, inp_ap, lens_ap):
    import concourse.mybir as mybir

    nc = tc.nc
    f32 = mybir.dt.float32
    bf16 = mybir.dt.bfloat16
    Alu = mybir.AluOpType
    Act = mybir.ActivationFunctionType

    cb0h, sb0h = math.cos(B_K[0] / 2), math.sin(B_K[0] / 2)
    cb1h, sb1h = math.cos(B_K[1] / 2), math.sin(B_K[1] / 2)
    cb2h, sb2h = math.cos(B_K[2] / 2), math.sin(B_K[2] / 2)
    cb0f, sb0f = math.cos(B_K[0]), math.sin(B_K[0])
    cb1f, sb1f = math.cos(B_K[1]), math.sin(B_K[1])

    def ttv(o, a, b, op):
        nc.vector.tensor_tensor(out=o, in0=a, in1=b, op=op)

    def ttp(o, a, b, op):
        nc.gpsimd.tensor_tensor(out=o, in0=a, in1=b, op=op)

    def stt(o, in0, scalar, in1, op0, op1):
        nc.vector.scalar_tensor_tensor(out=o, in0=in0, scalar=scalar, in1=in1,
                                       op0=op0, op1=op1)

    def ts(o, a, s1, s2=None):
        nc.scalar.activation(o, a, Act.Identity,
                             bias=(0.0 if s2 is None else cval(s2)), scale=s1)

    def ts_v(o, a, s1, s2=None):
        if s2 is None:
            nc.vector.tensor_scalar(out=o, in0=a, scalar1=s1, scalar2=None,
                                    op0=Alu.mult)
        else:
            nc.vector.tensor_scalar(out=o, in0=a, scalar1=s1, scalar2=s2,
                                    op0=Alu.mult, op1=Alu.add)

    def acopy(o, a):
        nc.scalar.activation(o, a, Act.Copy, bias=0.0, scale=1.0)

    # ------------------------------------------------------------------ pools
    persist = ctx.enter_context(tc.tile_pool(name="persist", bufs=1))
    Qp = [persist.tile([CB, L], f32, name=f"Qp_{c}") for c in range(4)]
    u0 = [persist.tile([CB, L], bf16, name=f"u0_{d}") for d in range(2)]
    u1 = [persist.tile([CB, L], bf16, name=f"u1_{d}") for d in range(3)]
    u2 = [persist.tile([CB, L], bf16, name=f"u2_{d}") for d in range(3)]
    w0 = [persist.tile([CB, L], bf16, name=f"w0_{d}") for d in range(3)]
    w1 = [persist.tile([CB, L], bf16, name=f"w1_{d}") for d in range(3)]
    w2 = [persist.tile([CB, L], f32, name=f"w2_{d}") for d in range(3)]
    cfb = [persist.tile([CB, L], bf16, name=f"cfb{i}") for i in range(3)]
    sfb = [persist.tile([CB, L], bf16, name=f"sfb{i}") for i in range(3)]
    out_sb = persist.tile([CB, 9 * L], f32, name="out_sb")
    ones = persist.tile([CB, NB], f32, name="ones")
    mask = persist.tile([CB, L], f32, name="mask")
    lens_sb = persist.tile([CB, 1], f32, name="lens_sb")

    nc.gpsimd.memset(ones[:], 1.0)
    nc.sync.dma_start(lens_sb[:], lens_ap)

    _consts = {}

    def cval(v):
        if v not in _consts:
            t = persist.tile([CB, 1], f32, name=f"cval_{len(_consts)}")
            nc.gpsimd.memset(t[:], v)
            _consts[v] = t[:]
        return _consts[v]

    # ------------------------------------------------------ phase A/B1 planes
    scan_pool = ctx.enter_context(tc.tile_pool(name="scan", bufs=1))
    tmp = [scan_pool.tile([CB, NL], f32, name=f"tmp_{i}") for i in range(16)]

    phase_b = tc.tile_pool(name="phase_b", bufs=1)
    pb = phase_b.__enter__()
    dih = pb.tile([CB, 3, L], f32, name="dih")
    for k in range(3):
        nc.sync.dma_start(dih[:][:, k, :], inp_ap[:, k, :])

    def bplane(name, dt_=f32):
        return pb.tile([CB, L], dt_, name=name)

    pang = [bplane(f"pang{k}") for k in range(3)]
    cf = [bplane(f"cf{i}") for i in range(3)]
    sf = [bplane(f"sf{i}") for i in range(3)]
    sq = bplane("sqtmp")
    sOh = bplane("sOh")
    ssum = bplane("ssum")
    sdif = bplane("sdif")
    cS = bplane("cS"); sS = bplane("sS")
    cD = bplane("cD"); sD = bplane("sD")
    cOh = bplane("cOh")
    q3 = [bplane(f"q3_{c}") for c in range(4)]
    qt = [bplane(f"qt_{c}") for c in range(4)]
    q4 = [bplane(f"q4_{c}") for c in range(4)]
    iota = bplane("iota")

    def qcombine(Lap, Rap, Oap, n):
        """O = L x R; comps w,x,y on DVE, comp z on Pool."""
        eng = [ttv, ttv, ttv, ttp]
        mv = []
        for k, (a, b) in enumerate(_QPAIRS):
            dst = tmp[k][:][:, 0:n]
            eng[k // 4](dst, Lap[a], Rap[b], Alu.mult)
            mv.append(dst)
        specs = [
            (0, 0, 1, Alu.subtract, 2, 3, Alu.add, Alu.subtract),
            (1, 4, 5, Alu.add, 6, 7, Alu.subtract, Alu.add),
            (2, 8, 9, Alu.subtract, 10, 11, Alu.add, Alu.add),
            (3, 12, 13, Alu.add, 15, 14, Alu.subtract, Alu.add),
        ]
        for comp, a, b, opab, c_, d_, opcd, opf in specs:
            e = eng[comp]
            e(mv[a], mv[a], mv[b], opab)
            e(mv[c_], mv[c_], mv[d_], opcd)
            e(Oap[comp], mv[a], mv[c_], opf)

    # --------- slab-pipelined: trig + rotor build + serial scan step ---------
    for i_slab in range(G):
        lo = i_slab * NB
        hi_ = lo + NB

        def S(p):
            return p[:][:, lo:hi_]

        for k, eng in enumerate((nc.vector, nc.gpsimd, nc.scalar)):
            psrc = dih[:][:, k, i_slab::G]
            if eng is nc.scalar:
                acopy(S(pang[k]), psrc)
            else:
                eng.tensor_copy(S(pang[k]), psrc)
        phi, psi, omg = S(pang[0]), S(pang[1]), S(pang[2])

        # trig (Sin on ACT; cos via 1-2sin^2(y/2), square+scale on DVE)
        nc.scalar.activation(S(sf[0]), phi, Act.Sin, bias=0.0, scale=1.0)
        nc.scalar.activation(S(sf[1]), psi, Act.Sin, bias=0.0, scale=1.0)
        nc.scalar.activation(S(sf[2]), omg, Act.Sin, bias=0.0, scale=1.0)
        for k, ang in enumerate((phi, psi, omg)):
            half = S(sOh) if k == 2 else S(sq)
            nc.scalar.activation(half, ang, Act.Sin, bias=0.0, scale=0.5)
            ttv(S(cf[k]), half, half, Alu.mult)
            ts_v(S(cf[k]), S(cf[k]), -2.0, 1.0)
        ttv(S(ssum), phi, psi, Alu.add)
        ttv(S(sdif), phi, psi, Alu.subtract)
        nc.scalar.activation(S(sS), S(ssum), Act.Sin, bias=0.0, scale=0.5)
        nc.scalar.activation(S(sD), S(sdif), Act.Sin, bias=0.0, scale=0.5)
        nc.scalar.activation(S(cS), S(ssum), Act.Sin, bias=0.0, scale=0.25)
        ttv(S(cS), S(cS), S(cS), Alu.mult)
        ts_v(S(cS), S(cS), -2.0, 1.0)
        nc.scalar.activation(S(cD), S(sdif), Act.Sin, bias=0.0, scale=0.25)
        ttv(S(cD), S(cD), S(cD), Alu.mult)
        ts_v(S(cD), S(cD), -2.0, 1.0)
        nc.scalar.activation(S(cOh), omg, Act.Sin, bias=cval(HALF_PI),
                             scale=0.5)

        # bf16 trig copies for the Pool/ACT u-vector build later
        for t_ in range(3):
            acopy(S(cfb[t_]), S(cf[t_]))
            acopy(S(sfb[t_]), S(sf[t_]))

        # q3 = qz(phi)qx(b0)qz(psi)qx(b1) directly from S/D trig
        ts_v(S(qt[0]), S(cD), sb0h * sb1h)
        stt(S(q3[0]), S(cS), cb0h * cb1h, S(qt[0]), Alu.mult, Alu.subtract)
        ts_v(S(qt[1]), S(cD), sb0h * cb1h)
        stt(S(q3[1]), S(cS), cb0h * sb1h, S(qt[1]), Alu.mult, Alu.add)
        ts_v(S(qt[2]), S(sS), cb0h * sb1h)
        stt(S(q3[2]), S(sD), sb0h * cb1h, S(qt[2]), Alu.mult, Alu.add)
        ts_v(S(qt[3]), S(sD), sb0h * sb1h)
        stt(S(q3[3]), S(sS), cb0h * cb1h, S(qt[3]), Alu.mult, Alu.subtract)

        # q4 = q3 * qz(omega/2)
        for c, (src, shuf, op) in enumerate((
                (q3[0], q3[3], Alu.subtract), (q3[1], q3[2], Alu.add),
                (q3[2], q3[1], Alu.subtract), (q3[3], q3[0], Alu.add))):
            e1, e2 = (ttv, ttp) if c % 2 else (ttp, ttv)
            e1(S(q4[c]), S(src), S(cOh), Alu.mult)
            e2(S(qt[c]), S(shuf), S(sOh), Alu.mult)
            ttv(S(q4[c]), S(q4[c]), S(qt[c]), op)

        # Q = q4 * qx(b2h) -> Qp slab
        ts_v(S(qt[0]), S(q4[1]), sb2h)
        stt(S(Qp[0]), S(q4[0]), cb2h, S(qt[0]), Alu.mult, Alu.subtract)
        ts_v(S(qt[1]), S(q4[0]), sb2h)
        stt(S(Qp[1]), S(q4[1]), cb2h, S(qt[1]), Alu.mult, Alu.add)
        ts_v(S(qt[2]), S(q4[3]), sb2h)
        stt(S(Qp[2]), S(q4[2]), cb2h, S(qt[2]), Alu.mult, Alu.add)
        ts_v(S(qt[3]), S(q4[2]), sb2h)
        stt(S(Qp[3]), S(q4[3]), cb2h, S(qt[3]), Alu.mult, Alu.subtract)

        # serial scan step: combine slab i-1 into slab i
        if i_slab > 0:
            Lap = [Qp[c][:][:, lo - NB:lo] for c in range(4)]
            Rap = [Qp[c][:][:, lo:hi_] for c in range(4)]
            qcombine(Lap, Rap, Rap, NB)

    # mask = (r < length); iota value r = G*j+i at permuted col i*NB+j
    nc.gpsimd.iota(iota[:], pattern=[[1, G], [G, NB]], base=0,
                   channel_multiplier=0, allow_small_or_imprecise_dtypes=True)
    nc.vector.tensor_scalar(out=mask[:], in0=iota[:], scalar1=lens_sb[:],
                            scalar2=None, op0=Alu.is_lt)

    # ---------------- Phase B2: u vectors (bf16; Pool + ScalarE) --------------
    p1 = scan_pool.tile([CB, L], bf16, name="p1")
    p2 = scan_pool.tile([CB, L], bf16, name="p2")
    p3 = scan_pool.tile([CB, L], bf16, name="p3")
    p4 = scan_pool.tile([CB, L], bf16, name="p4")
    ttp(p1[:], cfb[0][:], cfb[1][:], Alu.mult)
    ttp(p2[:], sfb[0][:], sfb[1][:], Alu.mult)
    ttp(p3[:], sfb[0][:], cfb[1][:], Alu.mult)
    ttp(p4[:], cfb[0][:], sfb[1][:], Alu.mult)

    v0 = [scan_pool.tile([CB, L], bf16, name=f"v0_{d}") for d in range(3)]
    bt1 = scan_pool.tile([CB, L], bf16, name="bt1")
    bt2 = scan_pool.tile([CB, L], bf16, name="bt2")
    ts(bt1[:], p2[:], -cb0f)
    ttp(v0[0][:], bt1[:], p1[:], Alu.add)
    ts(bt2[:], p4[:], cb0f)
    ttp(v0[1][:], bt2[:], p3[:], Alu.add)
    ts(v0[2][:], sfb[1][:], sb0f)

    ts(u0[0][:], cfb[0][:], R_KC[0])
    ts(u0[1][:], sfb[0][:], R_KC[0])
    nc.gpsimd.memset(u0[0][:][:, 0:1], 0.0)
    nc.gpsimd.memset(u0[1][:][:, 0:1], 0.0)

    ts(bt1[:], v0[0][:], R_KC[1])
    ttp(u1[0][:], bt1[:], u0[0][:], Alu.add)
    ts(bt2[:], v0[1][:], R_KC[1])
    ttp(u1[1][:], bt2[:], u0[1][:], Alu.add)
    ts(u1[2][:], v0[2][:], R_KC[1])

    c1x = scan_pool.tile([CB, L], bf16, name="c1x")
    c1y = scan_pool.tile([CB, L], bf16, name="c1y")
    c1z = scan_pool.tile([CB, L], bf16, name="c1z")
    ts(c1x[:], sfb[0][:], sb0f * sb1f)
    ts(bt1[:], p3[:], -cb0f * cb1f)
    ttp(c1x[:], bt1[:], c1x[:], Alu.add)
    ts(bt2[:], p4[:], -cb1f)
    ttp(c1x[:], bt2[:], c1x[:], Alu.add)
    ts(c1y[:], cfb[0][:], -sb0f * sb1f)
    ts(bt1[:], p1[:], cb0f * cb1f)
    ttp(c1y[:], bt1[:], c1y[:], Alu.add)
    ts(bt2[:], p2[:], -cb1f)
    ttp(c1y[:], bt2[:], c1y[:], Alu.add)
    ts(c1z[:], cfb[1][:], sb0f * cb1f, cb0f * sb1f)

    for d, c1 in enumerate((c1x, c1y, c1z)):
        qa = scan_pool.tile([CB, L], bf16, name=f"u2t_{d}")
        qb = scan_pool.tile([CB, L], bf16, name=f"u2s_{d}")
        ttp(qa[:], cfb[2][:], v0[d][:], Alu.mult)
        ttp(qb[:], sfb[2][:], c1[:], Alu.mult)
        ttp(qa[:], qa[:], qb[:], Alu.add)
        ts(qb[:], qa[:], R_KC[2])
        ttp(u2[d][:], qb[:], u1[d][:], Alu.add)

    # ---------------- L2: doubling scan over the NB block aggregates ----------
    s = 1
    while s < NB:
        base = (G - 1) * NB
        Lap = [Qp[c][:][:, base:base + NB - s] for c in range(4)]
        Rap = [Qp[c][:][:, base + s:base + NB] for c in range(4)]
        qcombine(Lap, Rap, Rap, NB - s)
        s *= 2

    phase_b.__exit__(None, None, None)

    # ---------------- prefix apply: Qp[0:NL] <- PQ (x) Qp[0:NL] ---------------
    rot_pool = ctx.enter_context(tc.tile_pool(name="rot", bufs=1))

    PQr = [rot_pool.tile([CB, NL], f32, name=f"PQr_{c}") for c in range(4)]
    for c in range(4):
        nc.vector.memset(PQr[c][:][:, 0:1], 1.0 if c == 0 else 0.0)
        acopy(PQr[c][:][:, 1:NB], Qp[c][:][:, (G - 1) * NB:G * NB - 1])
        seg = NB
        while seg < NL:
            acopy(PQr[c][:][:, seg:min(2 * seg, NL)],
                  PQr[c][:][:, 0:min(seg, NL - seg)])
            seg *= 2

    Lap = [PQr[c][:] for c in range(4)]
    Rap = [Qp[c][:][:, 0:NL] for c in range(4)]
    qcombine(Lap, Rap, Rap, NL)

    # Qex (bf16): cols [NB:L] = Qcum[0:NL]; cols [0:NB] = block prefix
    Qxb = [rot_pool.tile([CB, L], bf16, name=f"Qxb_{c}") for c in range(4)]
    for c in range(4):
        acopy(Qxb[c][:][:, 0:NB], PQr[c][:][:, 0:NB])
        acopy(Qxb[c][:][:, NB:L], Qp[c][:][:, 0:NL])

    # ---------------- single-stage rotation by Qex (bf16) ---------------------
    qw, qx, qy, qz = [Qxb[c][:] for c in range(4)]
    prod = {nm: rot_pool.tile([CB, L], bf16, name=f"pr_{nm}")
            for nm in _PROD_NAMES}
    cmb = {nm: rot_pool.tile([CB, L], bf16, name=f"cb_{nm}")
           for nm in _COMBO_NAMES}
    pr = {k: prod[k][:] for k in prod}
    ttv(pr["xx"], qx, qx, Alu.mult)
    ttv(pr["yy"], qy, qy, Alu.mult)
    ttp(pr["zz"], qz, qz, Alu.mult)
    ttv(pr["xy"], qx, qy, Alu.mult)
    ttv(pr["xz"], qx, qz, Alu.mult)
    ttp(pr["yz"], qy, qz, Alu.mult)
    ttv(pr["wx"], qw, qx, Alu.mult)
    ttv(pr["wy"], qw, qy, Alu.mult)
    ttv(pr["wz"], qw, qz, Alu.mult)
    ttv(cmb["S1"][:], pr["yy"], pr["zz"], Alu.add)
    ttv(cmb["S2"][:], pr["xx"], pr["zz"], Alu.add)
    ttp(cmb["S3"][:], pr["xx"], pr["yy"], Alu.add)
    ttv(cmb["A1"][:], pr["xy"], pr["wz"], Alu.add)
    ttv(cmb["D1"][:], pr["xy"], pr["wz"], Alu.subtract)
    ttv(cmb["A2"][:], pr["xz"], pr["wy"], Alu.add)
    ttp(cmb["D2"][:], pr["xz"], pr["wy"], Alu.subtract)
    ttv(cmb["A3"][:], pr["yz"], pr["wx"], Alu.add)
    ttv(cmb["D3"][:], pr["yz"], pr["wx"], Alu.subtract)

    rta = rot_pool.tile([CB, L], bf16, name="rta")
    rtb = rot_pool.tile([CB, L], bf16, name="rtb")
    rtc = rot_pool.tile([CB, L], bf16, name="rtc")

    def C(nm):
        return cmb[nm][:]

    def final_stt(out_ap, t_ap, base_ap):
        if base_ap is None:
            ts_v(out_ap, t_ap, 2.0)
        else:
            stt(out_ap, t_ap, 2.0, base_ap, Alu.mult, Alu.add)

    def rotate(uvec, wvec):
        vx, vy = uvec[0][:], uvec[1][:]
        vz = uvec[2][:] if len(uvec) > 2 else None
        ta, tb, tc_ = rta[:], rtb[:], rtc[:]
        ttv(ta, vy, C("D1"), Alu.mult)
        if vz is not None:
            ttp(tb, vz, C("A2"), Alu.mult)
            ttv(ta, ta, tb, Alu.add)
        ttv(tb, vx, C("S1"), Alu.mult)
        ttv(ta, ta, tb, Alu.subtract)
        final_stt(wvec[0][:], ta, vx)
        ttv(tb, vx, C("A1"), Alu.mult)
        if vz is not None:
            ttp(tc_, vz, C("D3"), Alu.mult)
            ttv(tb, tb, tc_, Alu.add)
        ttv(tc_, vy, C("S2"), Alu.mult)
        ttv(tb, tb, tc_, Alu.subtract)
        final_stt(wvec[1][:], tb, vy)
        ttv(tc_, vx, C("D2"), Alu.mult)
        ttp(ta, vy, C("A3"), Alu.mult)
        ttv(tc_, tc_, ta, Alu.add)
        if vz is not None:
            ttv(ta, vz, C("S3"), Alu.mult)
            ttv(tc_, tc_, ta, Alu.subtract)
            final_stt(wvec[2][:], tc_, vz)
        else:
            final_stt(wvec[2][:], tc_, None)

    rotate((u0[0], u0[1]), w0)
    rotate((u1[0], u1[1], u1[2]), w1)
    rotate((u2[0], u2[1], u2[2]), w2)

    # ---------------- Phase E: hierarchical cumsum + coords -------------------
    for i in range(1, G):
        for d in range(3):
            e = ttv if (i + d) % 4 else ttp
            e(w2[d][:][:, i * NB:(i + 1) * NB],
              w2[d][:][:, (i - 1) * NB:i * NB],
              w2[d][:][:, i * NB:(i + 1) * NB], Alu.add)

    Ot = [rot_pool.tile([CB, NB + 1], f32, name=f"Ot_{d}") for d in range(3)]
    Orr = [rot_pool.tile([CB, L], f32, name=f"Or_{d}") for d in range(3)]
    for d in range(3):
        nc.vector.memset(Ot[d][:][:, 0:1], 0.0)
        nc.vector.tensor_tensor_scan(
            out=Ot[d][:][:, 1:NB + 1],
            data0=ones[:],
            data1=w2[d][:][:, (G - 1) * NB:G * NB],
            initial=0.0, op0=Alu.mult, op1=Alu.add,
        )
        pl = Orr[d][:]
        acopy(pl[:, 0:NB], Ot[d][:][:, 0:NB])
        seg = NB
        while seg < L:
            acopy(pl[:, seg:min(2 * seg, L)], pl[:, 0:min(seg, L - seg)])
            seg *= 2

    Binc = [rot_pool.tile([CB, L], f32, name=f"Binc_{d}") for d in range(3)]
    Bex = [rot_pool.tile([CB, L], f32, name=f"Bex_{d}") for d in range(3)]
    for d in range(3):
        e = ttv if d != 1 else ttp
        e(Binc[d][:], w2[d][:], Orr[d][:], Alu.add)
        e(Bex[d][:][:, NB:L], w2[d][:][:, 0:NL], Orr[d][:][:, 0:NL], Alu.add)
        acopy(Bex[d][:][:, 0:NB], Ot[d][:][:, 0:NB])

    # coords in permuted layout; ScalarE un-permutes + interleaves into out_sb
    ct = [[rot_pool.tile([CB, L], f32, name=f"ct_{k}_{d}") for d in range(3)]
          for k in range(3)]
    for k, wk in enumerate((w0, w1, None)):
        for d in range(3):
            if k == 2:
                ttv(ct[k][d][:], Binc[d][:], mask[:], Alu.mult)
            else:
                e = ttv if (k + d) % 3 else ttp
                e(ct[k][d][:], wk[d][:], Bex[d][:], Alu.add)
                e2 = ttv if (k + d) % 2 else ttp
                e2(ct[k][d][:], ct[k][d][:], mask[:], Alu.mult)

    def outview(k, d):
        return out_sb[:].rearrange("p (j i q) -> p i j q", j=NB, i=G,
                                   q=9)[:, :, :, 3 * k + d]

    def pview(plane):
        return plane[:].rearrange("p (i j) -> p i j", j=NB)

    for k in range(3):
        for d in range(3):
            acopy(outview(k, d), pview(ct[k][d]))

    nc.sync.dma_start(out_ap, out_sb[:])


_CACHE = {}


def _build():
    from contextlib import ExitStack

    import concourse.bacc as bacc
    import concourse.mybir as mybir
    import concourse.tile as tile

    nc = bacc.Bacc("TRN2", target_bir_lowering=False, debug=False,
                   num_devices=N_CORES)
    inp = nc.dram_tensor("input", [CB, 3, L], mybir.dt.float32,
                         kind="ExternalInput").ap()
    lens = nc.dram_tensor("lens", [CB, 1], mybir.dt.float32,
                          kind="ExternalInput").ap()
    out = nc.dram_tensor("out", [CB, 9 * L], mybir.dt.float32,
                         kind="ExternalOutput").ap()
    with tile.TileContext(nc) as tc_ctx, ExitStack() as ctx:
        _body(ctx, tc_ctx, out, inp, lens)
    nc.compile()
    return nc


def get_nc():
    if "nc" not in _CACHE:
        _CACHE["nc"] = _build()
    return _CACHE["nc"]


def make_in_maps(input, angles_length):
    inp = np.ascontiguousarray(np.asarray(input, dtype=np.float32))
    lens = np.asarray(angles_length).astype(np.float32).reshape(B_FULL, 1)
    in_maps = []
    for i in range(N_CORES):
        sl = slice(i * CB, (i + 1) * CB)
        in_maps.append({
            "input": np.ascontiguousarray(inp[sl]),
            "lens": np.ascontiguousarray(lens[sl]),
        })
    return in_maps


def kernel(input, angles_length):
    from concourse.bass_utils import run_bass_kernel_spmd

    nc = get_nc()
    in_maps = make_in_maps(input, angles_length)
    res = run_bass_kernel_spmd(nc, in_maps, core_ids=list(range(N_CORES)))
    outs = [res.results[i]["out"] for i in range(N_CORES)]
    return np.concatenate(outs, axis=0).astype(np.float32)


# revision 17
# speedup vs baseline: 1.0414x; 1.0160x over previous
"""Trainium2 Bass kernel for nn_Angles2Backbone.

Full inputs:  input [1024, 3, 512] f32 (phi/psi/omega dihedrals), angles_length [1024] i64.
Full output:  [1024, 4608] f32 backbone coords (N, CA, C per residue, xyz interleaved).

Strategy: pure data parallelism — 128 protein chains per NeuronCore (batch on
partitions), 512 residues on the free axis.

Layout: residues are POSITION-MAJOR ("permuted"): residue r = G*j + i lives at
column i*NB + j (G=4 positions, NB=128 blocks).  Every scan step is then a
contiguous [128, NB] op and the block-prefix application is one contiguous
combine against a replicated prefix plane.

Pipeline (slab-pipelined start):
  per slab i: trig (ScalarE Sin) + residue rotor Q (DVE, f32) + serial scan
  step i (combines slab i-1 into slab i) — DVE never waits on a long serial
  ScalarE prefix.  Then: doubling scan over the 128 block aggregates (f32);
  intra-residue offsets u0/u1/u2 in bf16 on Pool/ScalarE (fills scan gaps);
  one contiguous prefix-apply combine; single-stage rotation by the exclusive
  cumulative rotor (bf16, errors are local); hierarchical cumsum (f32) and
  coordinate assembly, un-permuted by ScalarE interleave copies.
"""

import math

import numpy as np

N_CORES = 8
B_FULL = 1024
L = 512
CB = B_FULL // N_CORES  # 128 chains per core
NB = 128  # scan blocks
G = L // NB  # 4 positions per block
NL = L - NB  # 384

R_CA_C = 1.525
R_C_N = 1.330
R_N_CA = 1.460
CA_C_N = math.pi - 2.1186
C_N_CA = math.pi - 1.9391
N_CA_C = math.pi - 2.061

B_K = [C_N_CA, N_CA_C, CA_C_N]
R_KC = [R_C_N, R_N_CA, R_CA_C]

HALF_PI = math.pi / 2.0

_QPAIRS = [
    (0, 0), (1, 1), (2, 2), (3, 3),  # w
    (0, 1), (1, 0), (2, 3), (3, 2),  # x
    (0, 2), (1, 3), (2, 0), (3, 1),  # y
    (0, 3), (1, 2), (2, 1), (3, 0),  # z
]

_COMBO_NAMES = ("S1", "S2", "S3", "A1", "D1", "A2", "D2", "A3", "D3")
_PROD_NAMES = ("xx", "yy", "zz", "xy", "xz", "yz", "wx", "wy", "wz")


def _body(ctx, tc, out_ap, inp_ap, lens_ap):
    import concourse.mybir as mybir

    nc = tc.nc
    f32 = mybir.dt.float32
    bf16 = mybir.dt.bfloat16
    Alu = mybir.AluOpType
    Act = mybir.ActivationFunctionType

    cb0h, sb0h = math.cos(B_K[0] / 2), math.sin(B_K[0] / 2)
    cb1h, sb1h = math.cos(B_K[1] / 2), math.sin(B_K[1] / 2)
    cb2h, sb2h = math.cos(B_K[2] / 2), math.sin(B_K[2] / 2)
    cb0f, sb0f = math.cos(B_K[0]), math.sin(B_K[0])
    cb1f, sb1f = math.cos(B_K[1]), math.sin(B_K[1])

    def ttv(o, a, b, op):
        nc.vector.tensor_tensor(out=o, in0=a, in1=b, op=op)

    def ttp(o, a, b, op):
        nc.gpsimd.tensor_tensor(out=o, in0=a, in1=b, op=op)

    def stt(o, in0, scalar, in1, op0, op1):
        nc.vector.scalar_tensor_tensor(out=o, in0=in0, scalar=scalar, in1=in1,
                                       op0=op0, op1=op1)

    def ts(o, a, s1, s2=None):
        nc.scalar.activation(o, a, Act.Identity,
                             bias=(0.0 if s2 is None else cval(s2)), scale=s1)

    def ts_v(o, a, s1, s2=None):
        if s2 is None:
            nc.vector.tensor_scalar(out=o, in0=a, scalar1=s1, scalar2=None,
                                    op0=Alu.mult)
        else:
            nc.vector.tensor_scalar(out=o, in0=a, scalar1=s1, scalar2=s2,
                                    op0=Alu.mult, op1=Alu.add)

    def acopy(o, a):
        nc.scalar.activation(o, a, Act.Copy, bias=0.0, scale=1.0)

    # ------------------------------------------------------------------ pools
    persist = ctx.enter_context(tc.tile_pool(name="persist", bufs=1))
    Qp = [persist.tile([CB, L], f32, name=f"Qp_{c}") for c in range(4)]
    u0 = [persist.tile([CB, L], bf16, name=f"u0_{d}") for d in range(2)]
    u1 = [persist.tile([CB, L], bf16, name=f"u1_{d}") for d in range(3)]
    u2 = [persist.tile([CB, L], bf16, name=f"u2_{d}") for d in range(3)]
    w0 = [persist.tile([CB, L], bf16, name=f"w0_{d}") for d in range(3)]
    w1 = [persist.tile([CB, L], bf16, name=f"w1_{d}") for d in range(3)]
    w2 = [persist.tile([CB, L], f32, name=f"w2_{d}") for d in range(3)]
    cfb = [persist.tile([CB, L], bf16, name=f"cfb{i}") for i in range(3)]
    sfb = [persist.tile([CB, L], bf16, name=f"sfb{i}") for i in range(3)]
    out_sb = persist.tile([CB, 9 * L], f32, name="out_sb")
    ones = persist.tile([CB, NB], f32, name="ones")
    mask = persist.tile([CB, L], f32, name="mask")
    lens_sb = persist.tile([CB, 1], f32, name="lens_sb")

    nc.gpsimd.memset(ones[:], 1.0)
    nc.sync.dma_start(lens_sb[:], lens_ap)

    _consts = {}

    def cval(v):
        if v not in _consts:
            t = persist.tile([CB, 1], f32, name=f"cval_{len(_consts)}")
            nc.gpsimd.memset(t[:], v)
            _consts[v] = t[:]
        return _consts[v]

    # ------------------------------------------------------ phase A/B1 planes
    scan_pool = ctx.enter_context(tc.tile_pool(name="scan", bufs=1))
    tmp = [scan_pool.tile([CB, NL], f32, name=f"tmp_{i}") for i in range(16)]

    phase_b = tc.tile_pool(name="phase_b", bufs=1)
    pb = phase_b.__enter__()
    dih = pb.tile([CB, 3, L], f32, name="dih")
    for k in range(3):
        nc.sync.dma_start(dih[:][:, k, :], inp_ap[:, k, :])

    def bplane(name, dt_=f32):
        return pb.tile([CB, L], dt_, name=name)

    pang = [bplane(f"pang{k}") for k in range(3)]
    cf = [bplane(f"cf{i}") for i in range(3)]
    sf = [bplane(f"sf{i}") for i in range(3)]
    sq = bplane("sqtmp")
    sOh = bplane("sOh")
    ssum = bplane("ssum")
    sdif = bplane("sdif")
    cS = bplane("cS"); sS = bplane("sS")
    cD = bplane("cD"); sD = bplane("sD")
    cOh = bplane("cOh")
    q3 = [bplane(f"q3_{c}") for c in range(4)]
    qt = [bplane(f"qt_{c}") for c in range(4)]
    q4 = [bplane(f"q4_{c}") for c in range(4)]
    iota = bplane("iota")

    def qcombine(Lap, Rap, Oap, n):
        """O = L x R; comps w,x,y on DVE, comp z on Pool."""
        eng = [ttv, ttv, ttv, ttv]
        mv = []
        for k, (a, b) in enumerate(_QPAIRS):
            dst = tmp[k][:][:, 0:n]
            eng[k // 4](dst, Lap[a], Rap[b], Alu.mult)
            mv.append(dst)
        specs = [
            (0, 0, 1, Alu.subtract, 2, 3, Alu.add, Alu.subtract),
            (1, 4, 5, Alu.add, 6, 7, Alu.subtract, Alu.add),
            (2, 8, 9, Alu.subtract, 10, 11, Alu.add, Alu.add),
            (3, 12, 13, Alu.add, 15, 14, Alu.subtract, Alu.add),
        ]
        for comp, a, b, opab, c_, d_, opcd, opf in specs:
            e = eng[comp]
            e(mv[a], mv[a], mv[b], opab)
            e(mv[c_], mv[c_], mv[d_], opcd)
            e(Oap[comp], mv[a], mv[c_], opf)

    # --------- slab-pipelined: trig + rotor build + serial scan step ---------
    for i_slab in range(G):
        lo = i_slab * NB
        hi_ = lo + NB

        def S(p):
            return p[:][:, lo:hi_]

        for k in range(3):
            acopy(S(pang[k]), dih[:][:, k, i_slab::G])
        phi, psi, omg = S(pang[0]), S(pang[1]), S(pang[2])

        # trig (Sin on ACT; cos via 1-2sin^2(y/2), square+scale on DVE)
        nc.scalar.activation(S(sf[0]), phi, Act.Sin, bias=0.0, scale=1.0)
        nc.scalar.activation(S(sf[1]), psi, Act.Sin, bias=0.0, scale=1.0)
        nc.scalar.activation(S(sf[2]), omg, Act.Sin, bias=0.0, scale=1.0)
        for k, ang in enumerate((phi, psi, omg)):
            half = S(sOh) if k == 2 else S(sq)
            nc.scalar.activation(half, ang, Act.Sin, bias=0.0, scale=0.5)
            ttv(S(cf[k]), half, half, Alu.mult)
            ts_v(S(cf[k]), S(cf[k]), -2.0, 1.0)
        ttv(S(ssum), phi, psi, Alu.add)
        ttv(S(sdif), phi, psi, Alu.subtract)
        nc.scalar.activation(S(sS), S(ssum), Act.Sin, bias=0.0, scale=0.5)
        nc.scalar.activation(S(sD), S(sdif), Act.Sin, bias=0.0, scale=0.5)
        nc.scalar.activation(S(cS), S(ssum), Act.Sin, bias=0.0, scale=0.25)
        ttv(S(cS), S(cS), S(cS), Alu.mult)
        ts_v(S(cS), S(cS), -2.0, 1.0)
        nc.scalar.activation(S(cD), S(sdif), Act.Sin, bias=0.0, scale=0.25)
        ttv(S(cD), S(cD), S(cD), Alu.mult)
        ts_v(S(cD), S(cD), -2.0, 1.0)
        nc.scalar.activation(S(cOh), omg, Act.Sin, bias=cval(HALF_PI),
                             scale=0.5)

        # bf16 trig copies for the Pool/ACT u-vector build later
        for t_ in range(3):
            acopy(S(cfb[t_]), S(cf[t_]))
            acopy(S(sfb[t_]), S(sf[t_]))

        # q3 = qz(phi)qx(b0)qz(psi)qx(b1) directly from S/D trig
        ts_v(S(qt[0]), S(cD), sb0h * sb1h)
        stt(S(q3[0]), S(cS), cb0h * cb1h, S(qt[0]), Alu.mult, Alu.subtract)
        ts_v(S(qt[1]), S(cD), sb0h * cb1h)
        stt(S(q3[1]), S(cS), cb0h * sb1h, S(qt[1]), Alu.mult, Alu.add)
        ts_v(S(qt[2]), S(sS), cb0h * sb1h)
        stt(S(q3[2]), S(sD), sb0h * cb1h, S(qt[2]), Alu.mult, Alu.add)
        ts_v(S(qt[3]), S(sD), sb0h * sb1h)
        stt(S(q3[3]), S(sS), cb0h * cb1h, S(qt[3]), Alu.mult, Alu.subtract)

        # q4 = q3 * qz(omega/2)
        for c, (src, shuf, op) in enumerate((
                (q3[0], q3[3], Alu.subtract), (q3[1], q3[2], Alu.add),
                (q3[2], q3[1], Alu.subtract), (q3[3], q3[0], Alu.add))):
            ttv(S(q4[c]), S(src), S(cOh), Alu.mult)
            ttv(S(qt[c]), S(shuf), S(sOh), Alu.mult)
            ttv(S(q4[c]), S(q4[c]), S(qt[c]), op)

        # Q = q4 * qx(b2h) -> Qp slab
        ts_v(S(qt[0]), S(q4[1]), sb2h)
        stt(S(Qp[0]), S(q4[0]), cb2h, S(qt[0]), Alu.mult, Alu.subtract)
        ts_v(S(qt[1]), S(q4[0]), sb2h)
        stt(S(Qp[1]), S(q4[1]), cb2h, S(qt[1]), Alu.mult, Alu.add)
        ts_v(S(qt[2]), S(q4[3]), sb2h)
        stt(S(Qp[2]), S(q4[2]), cb2h, S(qt[2]), Alu.mult, Alu.add)
        ts_v(S(qt[3]), S(q4[2]), sb2h)
        stt(S(Qp[3]), S(q4[3]), cb2h, S(qt[3]), Alu.mult, Alu.subtract)

        # serial scan step: combine slab i-1 into slab i
        if i_slab > 0:
            Lap = [Qp[c][:][:, lo - NB:lo] for c in range(4)]
            Rap = [Qp[c][:][:, lo:hi_] for c in range(4)]
            qcombine(Lap, Rap, Rap, NB)

    # mask = (r < length); iota value r = G*j+i at permuted col i*NB+j
    nc.gpsimd.iota(iota[:], pattern=[[1, G], [G, NB]], base=0,
                   channel_multiplier=0, allow_small_or_imprecise_dtypes=True)
    nc.vector.tensor_scalar(out=mask[:], in0=iota[:], scalar1=lens_sb[:],
                            scalar2=None, op0=Alu.is_lt)

    # ---------------- Phase B2: u vectors (bf16; Pool + ScalarE) --------------
    p1 = scan_pool.tile([CB, L], bf16, name="p1")
    p2 = scan_pool.tile([CB, L], bf16, name="p2")
    p3 = scan_pool.tile([CB, L], bf16, name="p3")
    p4 = scan_pool.tile([CB, L], bf16, name="p4")
    ttv(p1[:], cfb[0][:], cfb[1][:], Alu.mult)
    ttv(p2[:], sfb[0][:], sfb[1][:], Alu.mult)
    ttv(p3[:], sfb[0][:], cfb[1][:], Alu.mult)
    ttv(p4[:], cfb[0][:], sfb[1][:], Alu.mult)

    v0 = [scan_pool.tile([CB, L], bf16, name=f"v0_{d}") for d in range(3)]
    bt1 = scan_pool.tile([CB, L], bf16, name="bt1")
    bt2 = scan_pool.tile([CB, L], bf16, name="bt2")
    ts(bt1[:], p2[:], -cb0f)
    ttv(v0[0][:], bt1[:], p1[:], Alu.add)
    ts(bt2[:], p4[:], cb0f)
    ttv(v0[1][:], bt2[:], p3[:], Alu.add)
    ts(v0[2][:], sfb[1][:], sb0f)

    ts(u0[0][:], cfb[0][:], R_KC[0])
    ts(u0[1][:], sfb[0][:], R_KC[0])
    nc.gpsimd.memset(u0[0][:][:, 0:1], 0.0)
    nc.gpsimd.memset(u0[1][:][:, 0:1], 0.0)

    ts(bt1[:], v0[0][:], R_KC[1])
    ttv(u1[0][:], bt1[:], u0[0][:], Alu.add)
    ts(bt2[:], v0[1][:], R_KC[1])
    ttv(u1[1][:], bt2[:], u0[1][:], Alu.add)
    ts(u1[2][:], v0[2][:], R_KC[1])

    c1x = scan_pool.tile([CB, L], bf16, name="c1x")
    c1y = scan_pool.tile([CB, L], bf16, name="c1y")
    c1z = scan_pool.tile([CB, L], bf16, name="c1z")
    ts(c1x[:], sfb[0][:], sb0f * sb1f)
    ts(bt1[:], p3[:], -cb0f * cb1f)
    ttv(c1x[:], bt1[:], c1x[:], Alu.add)
    ts(bt2[:], p4[:], -cb1f)
    ttv(c1x[:], bt2[:], c1x[:], Alu.add)
    ts(c1y[:], cfb[0][:], -sb0f * sb1f)
    ts(bt1[:], p1[:], cb0f * cb1f)
    ttv(c1y[:], bt1[:], c1y[:], Alu.add)
    ts(bt2[:], p2[:], -cb1f)
    ttv(c1y[:], bt2[:], c1y[:], Alu.add)
    ts(c1z[:], cfb[1][:], sb0f * cb1f, cb0f * sb1f)

    for d, c1 in enumerate((c1x, c1y, c1z)):
        qa = scan_pool.tile([CB, L], bf16, name=f"u2t_{d}")
        qb = scan_pool.tile([CB, L], bf16, name=f"u2s_{d}")
        ttv(qa[:], cfb[2][:], v0[d][:], Alu.mult)
        ttv(qb[:], sfb[2][:], c1[:], Alu.mult)
        ttv(qa[:], qa[:], qb[:], Alu.add)
        ts(qb[:], qa[:], R_KC[2])
        ttv(u2[d][:], qb[:], u1[d][:], Alu.add)

    # ---------------- L2: doubling scan over the NB block aggregates ----------
    s = 1
    while s < NB:
        base = (G - 1) * NB
        Lap = [Qp[c][:][:, base:base + NB - s] for c in range(4)]
        Rap = [Qp[c][:][:, base + s:base + NB] for c in range(4)]
        qcombine(Lap, Rap, Rap, NB - s)
        s *= 2

    phase_b.__exit__(None, None, None)

    # ---------------- prefix apply: Qp[0:NL] <- PQ (x) Qp[0:NL] ---------------
    rot_pool = ctx.enter_context(tc.tile_pool(name="rot", bufs=1))

    PQr = [rot_pool.tile([CB, NL], bf16, name=f"PQr_{c}") for c in range(4)]
    for c in range(4):
        nc.gpsimd.memset(PQr[c][:][:, 0:1], 1.0 if c == 0 else 0.0)
        acopy(PQr[c][:][:, 1:NB], Qp[c][:][:, (G - 1) * NB:G * NB - 1])
        seg = NB
        while seg < NL:
            acopy(PQr[c][:][:, seg:min(2 * seg, NL)],
                  PQr[c][:][:, 0:min(seg, NL - seg)])
            seg *= 2

    # Qex (bf16): cols [NB:L] get local prefixes, then the prefix-apply
    # combine runs in bf16 in place; cols [0:NB] = block prefix
    Qxb = [rot_pool.tile([CB, L], bf16, name=f"Qxb_{c}") for c in range(4)]
    btmp = [rot_pool.tile([CB, NL], bf16, name=f"btmp_{i}") for i in range(16)]
    for c in range(4):
        acopy(Qxb[c][:][:, 0:NB], PQr[c][:][:, 0:NB])
        acopy(Qxb[c][:][:, NB:L], Qp[c][:][:, 0:NL])

    def qcombine_bf(Lap, Rap, Oap, n):
        mv = []
        for k, (a, b) in enumerate(_QPAIRS):
            dst = btmp[k][:][:, 0:n]
            ttv(dst, Lap[a], Rap[b], Alu.mult)
            mv.append(dst)
        specs = [
            (0, 0, 1, Alu.subtract, 2, 3, Alu.add, Alu.subtract),
            (1, 4, 5, Alu.add, 6, 7, Alu.subtract, Alu.add),
            (2, 8, 9, Alu.subtract, 10, 11, Alu.add, Alu.add),
            (3, 12, 13, Alu.add, 15, 14, Alu.subtract, Alu.add),
        ]
        for comp, a, b, opab, c_, d_, opcd, opf in specs:
            ttv(mv[a], mv[a], mv[b], opab)
            ttv(mv[c_], mv[c_], mv[d_], opcd)
            ttv(Oap[comp], mv[a], mv[c_], opf)

    Lap = [PQr[c][:] for c in range(4)]
    Rap = [Qxb[c][:][:, NB:L] for c in range(4)]
    qcombine_bf(Lap, Rap, Rap, NL)

    # ---------------- single-stage rotation by Qex (bf16) ---------------------
    qw, qx, qy, qz = [Qxb[c][:] for c in range(4)]
    prod = {nm: rot_pool.tile([CB, L], bf16, name=f"pr_{nm}")
            for nm in _PROD_NAMES}
    cmb = {nm: rot_pool.tile([CB, L], bf16, name=f"cb_{nm}")
           for nm in _COMBO_NAMES}
    pr = {k: prod[k][:] for k in prod}
    ttv(pr["xx"], qx, qx, Alu.mult)
    ttv(pr["yy"], qy, qy, Alu.mult)
    ttv(pr["zz"], qz, qz, Alu.mult)
    ttv(pr["xy"], qx, qy, Alu.mult)
    ttv(pr["xz"], qx, qz, Alu.mult)
    ttv(pr["yz"], qy, qz, Alu.mult)
    ttv(pr["wx"], qw, qx, Alu.mult)
    ttv(pr["wy"], qw, qy, Alu.mult)
    ttv(pr["wz"], qw, qz, Alu.mult)
    ttv(cmb["S1"][:], pr["yy"], pr["zz"], Alu.add)
    ttv(cmb["S2"][:], pr["xx"], pr["zz"], Alu.add)
    ttv(cmb["S3"][:], pr["xx"], pr["yy"], Alu.add)
    ttv(cmb["A1"][:], pr["xy"], pr["wz"], Alu.add)
    ttv(cmb["D1"][:], pr["xy"], pr["wz"], Alu.subtract)
    ttv(cmb["A2"][:], pr["xz"], pr["wy"], Alu.add)
    ttv(cmb["D2"][:], pr["xz"], pr["wy"], Alu.subtract)
    ttv(cmb["A3"][:], pr["yz"], pr["wx"], Alu.add)
    ttv(cmb["D3"][:], pr["yz"], pr["wx"], Alu.subtract)

    rta = rot_pool.tile([CB, L], bf16, name="rta")
    rtb = rot_pool.tile([CB, L], bf16, name="rtb")
    rtc = rot_pool.tile([CB, L], bf16, name="rtc")

    def C(nm):
        return cmb[nm][:]

    def final_stt(out_ap, t_ap, base_ap):
        if base_ap is None:
            ts_v(out_ap, t_ap, 2.0)
        else:
            stt(out_ap, t_ap, 2.0, base_ap, Alu.mult, Alu.add)

    def rotate(uvec, wvec):
        vx, vy = uvec[0][:], uvec[1][:]
        vz = uvec[2][:] if len(uvec) > 2 else None
        ta, tb, tc_ = rta[:], rtb[:], rtc[:]
        ttv(ta, vy, C("D1"), Alu.mult)
        if vz is not None:
            ttv(tb, vz, C("A2"), Alu.mult)
            ttv(ta, ta, tb, Alu.add)
        ttv(tb, vx, C("S1"), Alu.mult)
        ttv(ta, ta, tb, Alu.subtract)
        final_stt(wvec[0][:], ta, vx)
        ttv(tb, vx, C("A1"), Alu.mult)
        if vz is not None:
            ttv(tc_, vz, C("D3"), Alu.mult)
            ttv(tb, tb, tc_, Alu.add)
        ttv(tc_, vy, C("S2"), Alu.mult)
        ttv(tb, tb, tc_, Alu.subtract)
        final_stt(wvec[1][:], tb, vy)
        ttv(tc_, vx, C("D2"), Alu.mult)
        ttv(ta, vy, C("A3"), Alu.mult)
        ttv(tc_, tc_, ta, Alu.add)
        if vz is not None:
            ttv(ta, vz, C("S3"), Alu.mult)
            ttv(tc_, tc_, ta, Alu.subtract)
            final_stt(wvec[2][:], tc_, vz)
        else:
            final_stt(wvec[2][:], tc_, None)

    rotate((u0[0], u0[1]), w0)
    rotate((u1[0], u1[1], u1[2]), w1)
    rotate((u2[0], u2[1], u2[2]), w2)

    # ---------------- Phase E: hierarchical cumsum + coords -------------------
    for i in range(1, G):
        for d in range(3):
            e = ttv if (i + d) % 4 else ttp
            e(w2[d][:][:, i * NB:(i + 1) * NB],
              w2[d][:][:, (i - 1) * NB:i * NB],
              w2[d][:][:, i * NB:(i + 1) * NB], Alu.add)

    Ot = [rot_pool.tile([CB, NB + 1], f32, name=f"Ot_{d}") for d in range(3)]
    Orr = [rot_pool.tile([CB, L], f32, name=f"Or_{d}") for d in range(3)]
    for d in range(3):
        nc.vector.memset(Ot[d][:][:, 0:1], 0.0)
        nc.vector.tensor_tensor_scan(
            out=Ot[d][:][:, 1:NB + 1],
            data0=ones[:],
            data1=w2[d][:][:, (G - 1) * NB:G * NB],
            initial=0.0, op0=Alu.mult, op1=Alu.add,
        )
        pl = Orr[d][:]
        acopy(pl[:, 0:NB], Ot[d][:][:, 0:NB])
        seg = NB
        while seg < L:
            acopy(pl[:, seg:min(2 * seg, L)], pl[:, 0:min(seg, L - seg)])
            seg *= 2

    Binc = [rot_pool.tile([CB, L], f32, name=f"Binc_{d}") for d in range(3)]
    Bex = [rot_pool.tile([CB, L], f32, name=f"Bex_{d}") for d in range(3)]
    for d in range(3):
        e = ttv if d != 1 else ttp
        e(Binc[d][:], w2[d][:], Orr[d][:], Alu.add)
        e(Bex[d][:][:, NB:L], w2[d][:][:, 0:NL], Orr[d][:][:, 0:NL], Alu.add)
        acopy(Bex[d][:][:, 0:NB], Ot[d][:][:, 0:NB])

    # coords in permuted layout; ScalarE un-permutes + interleaves into out_sb
    ct = [[rot_pool.tile([CB, L], f32, name=f"ct_{k}_{d}") for d in range(3)]
          for k in range(3)]
    for k, wk in enumerate((w0, w1, None)):
        for d in range(3):
            if k == 2:
                ttv(ct[k][d][:], Binc[d][:], mask[:], Alu.mult)
            else:
                e = ttv if (k + d) % 3 else ttp
                e(ct[k][d][:], wk[d][:], Bex[d][:], Alu.add)
                e2 = ttv if (k + d) % 2 else ttp
                e2(ct[k][d][:], ct[k][d][:], mask[:], Alu.mult)

    def outview(k, d):
        return out_sb[:].rearrange("p (j i q) -> p i j q", j=NB, i=G,
                                   q=9)[:, :, :, 3 * k + d]

    def pview(plane):
        return plane[:].rearrange("p (i j) -> p i j", j=NB)

    for k in range(3):
        for d in range(3):
            acopy(outview(k, d), pview(ct[k][d]))

    nc.sync.dma_start(out_ap, out_sb[:])


_CACHE = {}


def _build():
    from contextlib import ExitStack

    import concourse.bacc as bacc
    import concourse.mybir as mybir
    import concourse.tile as tile

    nc = bacc.Bacc("TRN2", target_bir_lowering=False, debug=False,
                   num_devices=N_CORES)
    inp = nc.dram_tensor("input", [CB, 3, L], mybir.dt.float32,
                         kind="ExternalInput").ap()
    lens = nc.dram_tensor("lens", [CB, 1], mybir.dt.float32,
                          kind="ExternalInput").ap()
    out = nc.dram_tensor("out", [CB, 9 * L], mybir.dt.float32,
                         kind="ExternalOutput").ap()
    with tile.TileContext(nc) as tc_ctx, ExitStack() as ctx:
        _body(ctx, tc_ctx, out, inp, lens)
    nc.compile()
    return nc


def get_nc():
    if "nc" not in _CACHE:
        _CACHE["nc"] = _build()
    return _CACHE["nc"]


def make_in_maps(input, angles_length):
    inp = np.ascontiguousarray(np.asarray(input, dtype=np.float32))
    lens = np.asarray(angles_length).astype(np.float32).reshape(B_FULL, 1)
    in_maps = []
    for i in range(N_CORES):
        sl = slice(i * CB, (i + 1) * CB)
        in_maps.append({
            "input": np.ascontiguousarray(inp[sl]),
            "lens": np.ascontiguousarray(lens[sl]),
        })
    return in_maps


def kernel(input, angles_length):
    from concourse.bass_utils import run_bass_kernel_spmd

    nc = get_nc()
    in_maps = make_in_maps(input, angles_length)
    res = run_bass_kernel_spmd(nc, in_maps, core_ids=list(range(N_CORES)))
    outs = [res.results[i]["out"] for i in range(N_CORES)]
    return np.concatenate(outs, axis=0).astype(np.float32)


# revision 20
# speedup vs baseline: 1.2160x; 1.1677x over previous
"""Trainium2 Bass kernel for nn_Angles2Backbone.

Full inputs:  input [1024, 3, 512] f32 (phi/psi/omega dihedrals), angles_length [1024] i64.
Full output:  [1024, 4608] f32 backbone coords (N, CA, C per residue, xyz interleaved).

Strategy: pure data parallelism — 128 protein chains per NeuronCore (batch on
partitions), 512 residues on the free axis, position-major permuted layout
(residue r = G*j + i at column i*NB + j; G=8 positions, NB=64 blocks).

All elementwise work runs on DVE (concurrent GpSimd tensor ops were measured to
poison DVE throughput ~2x via SBUF port contention); ScalarE carries trig,
copies, replication and the final un-permute; GpSimd only does iota/memsets.

Key throughput trick: the quaternion scan state is PACKED per slab as
[4 comps x NB] so one combine is 14 instructions instead of 28 — the 16
cross-component products collapse into 4 ops whose right operand uses
component-shuffle access patterns (affine, incl. negative strides), and the
add tree is partially fused across components.  The rotation stage is fused
across the three atom vectors (component-major u/w tiles).
"""

import math

import dataclasses

import numpy as np

N_CORES = 8
B_FULL = 1024
L = 512
CB = B_FULL // N_CORES
NB = 64   # scan blocks
G = L // NB  # 8 positions per block
NL = L - NB  # 448
ST = 4 * NB  # state width per slab (4 comps)

R_CA_C = 1.525
R_C_N = 1.330
R_N_CA = 1.460
CA_C_N = math.pi - 2.1186
C_N_CA = math.pi - 1.9391
N_CA_C = math.pi - 2.061

B_K = [C_N_CA, N_CA_C, CA_C_N]
R_KC = [R_C_N, R_N_CA, R_CA_C]

HALF_PI = math.pi / 2.0


def _mkap(base_ap, off, dims):
    """Raw AP from a tile's base [partition, free] AP: free dims replaced by
    `dims` ([stride, count] pairs, element units), offset advanced by `off`."""
    import concourse.mybir as mybir

    part = list(base_ap.ap[0])
    return dataclasses.replace(
        base_ap,
        offset=base_ap.offset + off,
        ap=mybir.VecI64Pair([part] + [list(d) for d in dims]),
    )


def _body(ctx, tc, out_ap, inp_ap, lens_ap):
    import concourse.mybir as mybir

    nc = tc.nc
    f32 = mybir.dt.float32
    bf16 = mybir.dt.bfloat16
    Alu = mybir.AluOpType
    Act = mybir.ActivationFunctionType

    cb0h, sb0h = math.cos(B_K[0] / 2), math.sin(B_K[0] / 2)
    cb1h, sb1h = math.cos(B_K[1] / 2), math.sin(B_K[1] / 2)
    cb2h, sb2h = math.cos(B_K[2] / 2), math.sin(B_K[2] / 2)
    cb0f, sb0f = math.cos(B_K[0]), math.sin(B_K[0])
    cb1f, sb1f = math.cos(B_K[1]), math.sin(B_K[1])

    def ttv(o, a, b, op):
        nc.vector.tensor_tensor(out=o, in0=a, in1=b, op=op)

    def stt(o, in0, scalar, in1, op0, op1):
        nc.vector.scalar_tensor_tensor(out=o, in0=in0, scalar=scalar, in1=in1,
                                       op0=op0, op1=op1)

    def ts(o, a, s1, s2=None):
        nc.scalar.activation(o, a, Act.Identity,
                             bias=(0.0 if s2 is None else cval(s2)), scale=s1)

    def ts_v(o, a, s1, s2=None):
        if s2 is None:
            nc.vector.tensor_scalar(out=o, in0=a, scalar1=s1, scalar2=None,
                                    op0=Alu.mult)
        else:
            nc.vector.tensor_scalar(out=o, in0=a, scalar1=s1, scalar2=s2,
                                    op0=Alu.mult, op1=Alu.add)

    def acopy(o, a):
        nc.scalar.activation(o, a, Act.Copy, bias=0.0, scale=1.0)

    # ------------------------------------------------------------------ pools
    persist = ctx.enter_context(tc.tile_pool(name="persist", bufs=1))
    Qs = persist.tile([CB, G * ST], f32, name="Qs")  # packed scan state
    # component-major u/w tiles: [:, k, :] = vector k's component plane
    UX = persist.tile([CB, 3, L], bf16, name="UX")
    UY = persist.tile([CB, 3, L], bf16, name="UY")
    UZ = persist.tile([CB, 3, L], bf16, name="UZ")
    WX = persist.tile([CB, 3, L], bf16, name="WX")
    WY = persist.tile([CB, 3, L], bf16, name="WY")
    WZ = persist.tile([CB, 3, L], bf16, name="WZ")
    cfb = [persist.tile([CB, L], bf16, name=f"cfb{i}") for i in range(3)]
    sfb = [persist.tile([CB, L], bf16, name=f"sfb{i}") for i in range(3)]
    out_sb = persist.tile([CB, 9 * L], f32, name="out_sb")
    ones = persist.tile([CB, NB], f32, name="ones")
    mask = persist.tile([CB, L], f32, name="mask")
    lens_sb = persist.tile([CB, 1], f32, name="lens_sb")

    nc.gpsimd.memset(ones[:], 1.0)
    nc.sync.dma_start(lens_sb[:], lens_ap)

    _consts = {}

    def cval(v):
        if v not in _consts:
            t = persist.tile([CB, 1], f32, name=f"cval_{len(_consts)}")
            nc.gpsimd.memset(t[:], v)
            _consts[v] = t[:]
        return _consts[v]

    # scan temporaries (packed product/accumulator planes)
    scan_pool = ctx.enter_context(tc.tile_pool(name="scan", bufs=1))
    Pf = scan_pool.tile([CB, 16 * NB], f32, name="Pf")
    Af = scan_pool.tile([CB, 4 * NB], f32, name="Af")
    Pb = scan_pool.tile([CB, 16 * NL], bf16, name="Pb")
    Ab = scan_pool.tile([CB, 4 * NL], bf16, name="Ab")

    def qcombine(Lt, Lbase, Lc, Rt, Rbase, Rc, Ot, Obase, Oc, n, Pt, At):
        """Packed-state quaternion combine: O = L (x) R.
        (tile, base, comp-stride) triples; n = active columns; P/A temps."""
        Lb, Rb, Ob = Lt[:], Rt[:], Ot[:]
        P, A = Pt[:], At[:]

        def nat4(t, b, C):
            return _mkap(t, b, [[C, 4], [1, n]])

        def nat22(t, b, C):
            return _mkap(t, b, [[2 * C, 2], [C, 2], [1, n]])

        sig = {
            0: lambda t, b, C: nat22(t, b, C),
            1: lambda t, b, C: _mkap(t, b + C, [[2 * C, 2], [-C, 2], [1, n]]),
            2: lambda t, b, C: _mkap(t, b + 2 * C, [[-2 * C, 2], [C, 2], [1, n]]),
            3: lambda t, b, C: _mkap(t, b + 3 * C, [[-C, 4], [1, n]]),
        }
        # products: P[g] (4 cells, L-comp order) = L_nat * R_sigma(g)
        for g in range(4):
            dst = _mkap(P, g * 4 * n, [[2 * n, 2], [n, 2], [1, n]])
            ttv(dst, nat22(Lb, Lbase, Lc), sig[g](Rb, Rbase, Rc), Alu.mult)
        # add tree (cells: group g at 4gn, cell p at +pn)
        cell = lambda g, p: P[:, (4 * g + p) * n:(4 * g + p + 1) * n]
        ag = lambda g: A[:, g * n:(g + 1) * n]
        # level 1 pairs
        ttv(ag(0), cell(0, 1), cell(0, 2), Alu.add)
        ttv(ag(2), cell(2, 2), cell(2, 3), Alu.add)
        ttv(_mkap(A, n, [[2 * n, 2], [1, n]]),
            _mkap(P, 4 * n, [[8 * n, 2], [1, n]]),
            _mkap(P, 5 * n, [[8 * n, 2], [1, n]]), Alu.add)
        # level 2 (third terms): w&z share cell pos 3
        ttv(_mkap(A, 0, [[3 * n, 2], [1, n]]),
            _mkap(A, 0, [[3 * n, 2], [1, n]]),
            _mkap(P, 3 * n, [[12 * n, 2], [1, n]]), Alu.add)
        ttv(ag(1), ag(1), cell(1, 2), Alu.add)
        ttv(ag(2), ag(2), cell(2, 0), Alu.add)
        # level 3 (isolated +- sum) -> state comps
        oc = lambda c: _mkap(Ob, Obase + c * Oc, [[1, n]])
        ttv(oc(0), cell(0, 0), ag(0), Alu.subtract)
        ttv(oc(1), ag(1), cell(1, 3), Alu.subtract)
        ttv(oc(2), ag(2), cell(2, 1), Alu.subtract)
        ttv(oc(3), ag(3), cell(3, 2), Alu.subtract)

    # ------------------------------------------------------ phase A/B1 planes
    phase_b = tc.tile_pool(name="phase_b", bufs=1)
    pb = phase_b.__enter__()
    dih = pb.tile([CB, 3, L], f32, name="dih")
    for k in range(3):
        nc.sync.dma_start(dih[:][:, k, :], inp_ap[:, k, :])

    def bplane(name, dt_=f32):
        return pb.tile([CB, L], dt_, name=name)

    pang = [bplane(f"pang{k}") for k in range(3)]
    cf = [bplane(f"cf{i}") for i in range(3)]
    sf = [bplane(f"sf{i}") for i in range(3)]
    sq = bplane("sqtmp")
    sOh = bplane("sOh")
    ssum = bplane("ssum")
    sdif = bplane("sdif")
    cS = bplane("cS"); sS = bplane("sS")
    cD = bplane("cD"); sD = bplane("sD")
    cOh = bplane("cOh")
    q3 = [bplane(f"q3_{c}") for c in range(4)]
    qt = [bplane(f"qt_{c}") for c in range(4)]
    q4 = [bplane(f"q4_{c}") for c in range(4)]
    iota = bplane("iota")

    # B1 in two halves (cols [0:256], [256:512]) so ScalarE trig pipelines
    # with the DVE rotor build; L1 scan steps interleave after.
    HW_ = L // 2

    for h in range(2):
        lo = h * HW_
        hi_ = lo + HW_

        def S(p):
            return p[:][:, lo:hi_]

        for k in range(3):
            # permuted: pang[col i*NB+j] = dih[col G*j+i]; half h = slabs 4h..
            src = _mkap(dih[:], k * L + 4 * h,
                        [[1, 4], [G, NB]])
            acopy(S(pang[k]).rearrange("p (a b) -> p a b", b=NB), src)
        phi, psi, omg = S(pang[0]), S(pang[1]), S(pang[2])

        nc.scalar.activation(S(sf[0]), phi, Act.Sin, bias=0.0, scale=1.0)
        nc.scalar.activation(S(sf[1]), psi, Act.Sin, bias=0.0, scale=1.0)
        nc.scalar.activation(S(sf[2]), omg, Act.Sin, bias=0.0, scale=1.0)
        for k, ang in enumerate((phi, psi, omg)):
            half = S(sOh) if k == 2 else S(sq)
            nc.scalar.activation(half, ang, Act.Sin, bias=0.0, scale=0.5)
            ttv(S(cf[k]), half, half, Alu.mult)
            ts_v(S(cf[k]), S(cf[k]), -2.0, 1.0)
        ttv(S(ssum), phi, psi, Alu.add)
        ttv(S(sdif), phi, psi, Alu.subtract)
        nc.scalar.activation(S(sS), S(ssum), Act.Sin, bias=0.0, scale=0.5)
        nc.scalar.activation(S(sD), S(sdif), Act.Sin, bias=0.0, scale=0.5)
        nc.scalar.activation(S(cS), S(ssum), Act.Sin, bias=0.0, scale=0.25)
        ttv(S(cS), S(cS), S(cS), Alu.mult)
        ts_v(S(cS), S(cS), -2.0, 1.0)
        nc.scalar.activation(S(cD), S(sdif), Act.Sin, bias=0.0, scale=0.25)
        ttv(S(cD), S(cD), S(cD), Alu.mult)
        ts_v(S(cD), S(cD), -2.0, 1.0)
        nc.scalar.activation(S(cOh), omg, Act.Sin, bias=cval(HALF_PI),
                             scale=0.5)

        for t_ in range(3):
            acopy(S(cfb[t_]), S(cf[t_]))
            acopy(S(sfb[t_]), S(sf[t_]))

        # q3 = qz(phi)qx(b0)qz(psi)qx(b1) from S/D trig
        ts_v(S(qt[0]), S(cD), sb0h * sb1h)
        stt(S(q3[0]), S(cS), cb0h * cb1h, S(qt[0]), Alu.mult, Alu.subtract)
        ts_v(S(qt[1]), S(cD), sb0h * cb1h)
        stt(S(q3[1]), S(cS), cb0h * sb1h, S(qt[1]), Alu.mult, Alu.add)
        ts_v(S(qt[2]), S(sS), cb0h * sb1h)
        stt(S(q3[2]), S(sD), sb0h * cb1h, S(qt[2]), Alu.mult, Alu.add)
        ts_v(S(qt[3]), S(sD), sb0h * sb1h)
        stt(S(q3[3]), S(sS), cb0h * cb1h, S(qt[3]), Alu.mult, Alu.subtract)

        # q4 = q3 * qz(omega/2)
        for c, (src_, shuf, op) in enumerate((
                (q3[0], q3[3], Alu.subtract), (q3[1], q3[2], Alu.add),
                (q3[2], q3[1], Alu.subtract), (q3[3], q3[0], Alu.add))):
            ttv(S(q4[c]), S(src_), S(cOh), Alu.mult)
            ttv(S(qt[c]), S(shuf), S(sOh), Alu.mult)
            ttv(S(q4[c]), S(q4[c]), S(qt[c]), op)

        # Q = q4 * qx(b2h) -> packed state: slab s comp c at s*ST + c*NB
        def stview(c):
            return _mkap(Qs[:], 4 * h * ST + c * NB, [[ST, 4], [1, NB]])

        def hview(p):
            return S(p).rearrange("p (a b) -> p a b", b=NB)

        ts_v(S(qt[0]), S(q4[1]), sb2h)
        stt(stview(0), hview(q4[0]), cb2h, hview(qt[0]), Alu.mult, Alu.subtract)
        ts_v(S(qt[1]), S(q4[0]), sb2h)
        stt(stview(1), hview(q4[1]), cb2h, hview(qt[1]), Alu.mult, Alu.add)
        ts_v(S(qt[2]), S(q4[3]), sb2h)
        stt(stview(2), hview(q4[2]), cb2h, hview(qt[2]), Alu.mult, Alu.add)
        ts_v(S(qt[3]), S(q4[2]), sb2h)
        stt(stview(3), hview(q4[3]), cb2h, hview(qt[3]), Alu.mult, Alu.subtract)

        # L1 serial scan steps available after this half
        for i in range(max(1, 4 * h), 4 * h + 4):
            qcombine(Qs, (i - 1) * ST, NB, Qs, i * ST, NB, Qs, i * ST, NB,
                     NB, Pf, Af)

    # mask = (r < length); iota value r = G*j+i at permuted col i*NB+j
    nc.gpsimd.iota(iota[:], pattern=[[1, G], [G, NB]], base=0,
                   channel_multiplier=0, allow_small_or_imprecise_dtypes=True)
    nc.vector.tensor_scalar(out=mask[:], in0=iota[:], scalar1=lens_sb[:],
                            scalar2=None, op0=Alu.is_lt)

    # ---------------- Phase B2: u vectors (bf16; mostly ScalarE+DVE) ----------
    def uview(t, k):
        return t[:][:, k, :]

    p1 = scan_pool.tile([CB, L], bf16, name="p1")
    p2 = scan_pool.tile([CB, L], bf16, name="p2")
    p3 = scan_pool.tile([CB, L], bf16, name="p3")
    p4 = scan_pool.tile([CB, L], bf16, name="p4")
    ttv(p1[:], cfb[0][:], cfb[1][:], Alu.mult)
    ttv(p2[:], sfb[0][:], sfb[1][:], Alu.mult)
    ttv(p3[:], sfb[0][:], cfb[1][:], Alu.mult)
    ttv(p4[:], cfb[0][:], sfb[1][:], Alu.mult)

    v0 = [scan_pool.tile([CB, L], bf16, name=f"v0_{d}") for d in range(3)]
    bt1 = scan_pool.tile([CB, L], bf16, name="bt1")
    bt2 = scan_pool.tile([CB, L], bf16, name="bt2")
    ts(bt1[:], p2[:], -cb0f)
    ttv(v0[0][:], bt1[:], p1[:], Alu.add)
    ts(bt2[:], p4[:], cb0f)
    ttv(v0[1][:], bt2[:], p3[:], Alu.add)
    ts(v0[2][:], sfb[1][:], sb0f)

    ts(uview(UX, 0), cfb[0][:], R_KC[0])
    ts(uview(UY, 0), sfb[0][:], R_KC[0])
    nc.gpsimd.memset(UZ[:][:, 0, :], 0.0)
    nc.gpsimd.memset(UX[:][:, 0, 0:1], 0.0)
    nc.gpsimd.memset(UY[:][:, 0, 0:1], 0.0)

    ts(bt1[:], v0[0][:], R_KC[1])
    ttv(uview(UX, 1), bt1[:], uview(UX, 0), Alu.add)
    ts(bt2[:], v0[1][:], R_KC[1])
    ttv(uview(UY, 1), bt2[:], uview(UY, 0), Alu.add)
    ts(uview(UZ, 1), v0[2][:], R_KC[1])

    c1x = scan_pool.tile([CB, L], bf16, name="c1x")
    c1y = scan_pool.tile([CB, L], bf16, name="c1y")
    c1z = scan_pool.tile([CB, L], bf16, name="c1z")
    ts(c1x[:], sfb[0][:], sb0f * sb1f)
    ts(bt1[:], p3[:], -cb0f * cb1f)
    ttv(c1x[:], bt1[:], c1x[:], Alu.add)
    ts(bt2[:], p4[:], -cb1f)
    ttv(c1x[:], bt2[:], c1x[:], Alu.add)
    ts(c1y[:], cfb[0][:], -sb0f * sb1f)
    ts(bt1[:], p1[:], cb0f * cb1f)
    ttv(c1y[:], bt1[:], c1y[:], Alu.add)
    ts(bt2[:], p2[:], -cb1f)
    ttv(c1y[:], bt2[:], c1y[:], Alu.add)
    ts(c1z[:], cfb[1][:], sb0f * cb1f, cb0f * sb1f)

    for d, (c1, UT) in enumerate(((c1x, UX), (c1y, UY), (c1z, UZ))):
        qa = scan_pool.tile([CB, L], bf16, name=f"u2t_{d}")
        qb = scan_pool.tile([CB, L], bf16, name=f"u2s_{d}")
        ttv(qa[:], cfb[2][:], v0[d][:], Alu.mult)
        ttv(qb[:], sfb[2][:], c1[:], Alu.mult)
        ttv(qa[:], qa[:], qb[:], Alu.add)
        ts(qb[:], qa[:], R_KC[2])
        ttv(uview(UT, 2), qb[:], uview(UT, 1), Alu.add)

    # ---------------- L2: doubling scan over the NB block aggregates ----------
    s = 1
    while s < NB:
        base = (G - 1) * ST
        qcombine(Qs, base, NB, Qs, base + s, NB, Qs, base + s, NB,
                 NB - s, Pf, Af)
        s *= 2

    phase_b.__exit__(None, None, None)

    # ---------------- prefix apply (bf16, packed) -----------------------------
    rot_pool = ctx.enter_context(tc.tile_pool(name="rot", bufs=1))

    # PQr state [CB, 4 x NL] bf16: exclusive block prefix replicated over the
    # G-1 applied slabs; Qxb state [CB, 4 x L] bf16: local prefixes then the
    # in-place apply makes it the exclusive cumulative rotor per residue.
    PQr = rot_pool.tile([CB, 4 * NL], bf16, name="PQr")
    Qxb = rot_pool.tile([CB, 4 * L], bf16, name="Qxb")
    aggbase = (G - 1) * ST
    for c in range(4):
        nc.gpsimd.memset(PQr[:][:, c * NL:c * NL + 1], 1.0 if c == 0 else 0.0)
        acopy(PQr[:][:, c * NL + 1:c * NL + NB],
              Qs[:][:, aggbase + c * NB:aggbase + c * NB + NB - 1])
        seg = NB
        while seg < NL:
            acopy(PQr[:][:, c * NL + seg:c * NL + min(2 * seg, NL)],
                  PQr[:][:, c * NL:c * NL + min(seg, NL - seg)])
            seg *= 2
        # local prefixes into Qxb cols [NB:L]; block prefix into cols [0:NB]
        acopy(Qxb[:][:, c * L:c * L + NB], PQr[:][:, c * NL:c * NL + NB])
        lview = _mkap(Qxb[:], c * L + NB, [[NB, G - 1], [1, NB]])
        sview = _mkap(Qs[:], c * NB, [[ST, G - 1], [1, NB]])
        acopy(lview, sview)

    qcombine(PQr, 0, NL, Qxb, NB, L, Qxb, NB, L, NL, Pb, Ab)

    # ---------------- fused rotation by Qex -----------------------------------
    # products: xx|yy|zz, xy|xz, wx|wy|wz, yz
    Pr = rot_pool.tile([CB, 6 * L], bf16, name="Pr")   # xx yy zz xy xz yz
    Wp = rot_pool.tile([CB, 3 * L], bf16, name="Wp")   # wx wy wz
    Scm = rot_pool.tile([CB, 3 * L], bf16, name="Scm")  # S1 S2 S3
    Acm = rot_pool.tile([CB, 3 * L], bf16, name="Acm")  # A1 A2 A3
    Dcm = rot_pool.tile([CB, 3 * L], bf16, name="Dcm")  # D1 D2 D3

    def qc(c):  # Qxb comp plane
        return Qxb[:][:, c * L:(c + 1) * L]

    def seg3(t, i, m=1):
        return t[:][:, i * L:(i + m) * L]

    ttv(seg3(Pr, 0, 3).rearrange("p (a b) -> p a b", b=L),
        _mkap(Qxb[:], L, [[L, 3], [1, L]]),
        _mkap(Qxb[:], L, [[L, 3], [1, L]]), Alu.mult)
    ttv(seg3(Pr, 3, 2).rearrange("p (a b) -> p a b", b=L),
        _mkap(Qxb[:], L, [[0, 2], [1, L]]),
        _mkap(Qxb[:], 2 * L, [[L, 2], [1, L]]), Alu.mult)
    ttv(seg3(Pr, 5, 1), qc(2), qc(3), Alu.mult)
    ttv(Wp[:].rearrange("p (a b) -> p a b", b=L),
        _mkap(Qxb[:], 0, [[0, 3], [1, L]]),
        _mkap(Qxb[:], L, [[L, 3], [1, L]]), Alu.mult)
    # combos: S1=yy+zz; (S2,S3)=xx+(zz,yy); A/D = (xy,xz,yz) -+ (wz,wy,wx)
    ttv(seg3(Scm, 0, 1), seg3(Pr, 1, 1), seg3(Pr, 2, 1), Alu.add)
    ttv(seg3(Scm, 1, 2).rearrange("p (a b) -> p a b", b=L),
        _mkap(Pr[:], 0, [[0, 2], [1, L]]),
        _mkap(Pr[:], 2 * L, [[-L, 2], [1, L]]), Alu.add)
    ttv(Acm[:].rearrange("p (a b) -> p a b", b=L),
        _mkap(Pr[:], 3 * L, [[L, 3], [1, L]]),
        _mkap(Wp[:], 2 * L, [[-L, 3], [1, L]]), Alu.add)
    ttv(Dcm[:].rearrange("p (a b) -> p a b", b=L),
        _mkap(Pr[:], 3 * L, [[L, 3], [1, L]]),
        _mkap(Wp[:], 2 * L, [[-L, 3], [1, L]]), Alu.subtract)

    rt1 = rot_pool.tile([CB, 3 * L], bf16, name="rt1")
    rt2 = rot_pool.tile([CB, 3 * L], bf16, name="rt2")

    def cb3(t, i):  # combo i broadcast over the 3 atom vectors
        return _mkap(t[:], i * L, [[0, 3], [1, L]])

    def full3(t):
        ap = t[:]
        if len(ap.shape) == 3:
            return ap
        return ap.rearrange("p (a b) -> p a b", b=L)

    # w_x = ux + 2(uy*D1 + uz*A2 - ux*S1)
    ttv(full3(rt1), full3(UY), cb3(Dcm, 0), Alu.mult)
    ttv(full3(rt2), full3(UZ), cb3(Acm, 1), Alu.mult)
    ttv(full3(rt1), full3(rt1), full3(rt2), Alu.add)
    ttv(full3(rt2), full3(UX), cb3(Scm, 0), Alu.mult)
    ttv(full3(rt1), full3(rt1), full3(rt2), Alu.subtract)
    stt(full3(WX), full3(rt1), 2.0, full3(UX), Alu.mult, Alu.add)
    # w_y = uy + 2(ux*A1 + uz*D3 - uy*S2)
    ttv(full3(rt1), full3(UX), cb3(Acm, 0), Alu.mult)
    ttv(full3(rt2), full3(UZ), cb3(Dcm, 2), Alu.mult)
    ttv(full3(rt1), full3(rt1), full3(rt2), Alu.add)
    ttv(full3(rt2), full3(UY), cb3(Scm, 1), Alu.mult)
    ttv(full3(rt1), full3(rt1), full3(rt2), Alu.subtract)
    stt(full3(WY), full3(rt1), 2.0, full3(UY), Alu.mult, Alu.add)
    # w_z = uz + 2(ux*D2 + uy*A3 - uz*S3)
    ttv(full3(rt1), full3(UX), cb3(Dcm, 1), Alu.mult)
    ttv(full3(rt2), full3(UY), cb3(Acm, 2), Alu.mult)
    ttv(full3(rt1), full3(rt1), full3(rt2), Alu.add)
    ttv(full3(rt2), full3(UZ), cb3(Scm, 2), Alu.mult)
    ttv(full3(rt1), full3(rt1), full3(rt2), Alu.subtract)
    stt(full3(WZ), full3(rt1), 2.0, full3(UZ), Alu.mult, Alu.add)

    # ---------------- Phase E: hierarchical cumsum + coords -------------------
    dpl = [rot_pool.tile([CB, L], f32, name=f"dpl_{d}") for d in range(3)]
    for d, WT in enumerate((WX, WY, WZ)):
        acopy(dpl[d][:], WT[:][:, 2, :])
    for i in range(1, G):
        for d in range(3):
            ttv(dpl[d][:][:, i * NB:(i + 1) * NB],
                dpl[d][:][:, (i - 1) * NB:i * NB],
                dpl[d][:][:, i * NB:(i + 1) * NB], Alu.add)

    Ot = [rot_pool.tile([CB, NB + 1], f32, name=f"Ot_{d}") for d in range(3)]
    Orr = [rot_pool.tile([CB, L], f32, name=f"Or_{d}") for d in range(3)]
    for d in range(3):
        nc.vector.memset(Ot[d][:][:, 0:1], 0.0)
        nc.vector.tensor_tensor_scan(
            out=Ot[d][:][:, 1:NB + 1],
            data0=ones[:],
            data1=dpl[d][:][:, (G - 1) * NB:G * NB],
            initial=0.0, op0=Alu.mult, op1=Alu.add,
        )
        pl = Orr[d][:]
        acopy(pl[:, 0:NB], Ot[d][:][:, 0:NB])
        seg = NB
        while seg < L:
            acopy(pl[:, seg:min(2 * seg, L)], pl[:, 0:min(seg, L - seg)])
            seg *= 2

    Binc = [rot_pool.tile([CB, L], f32, name=f"Binc_{d}") for d in range(3)]
    Bex = [rot_pool.tile([CB, L], bf16, name=f"Bex_{d}") for d in range(3)]
    maskb = rot_pool.tile([CB, L], bf16, name="maskb")
    acopy(maskb[:], mask[:])
    for d in range(3):
        ttv(Binc[d][:], dpl[d][:], Orr[d][:], Alu.add)
        ttv(Bex[d][:][:, NB:L], dpl[d][:][:, 0:NL], Orr[d][:][:, 0:NL],
            Alu.add)
        acopy(Bex[d][:][:, 0:NB], Ot[d][:][:, 0:NB])

    ct = [[rot_pool.tile([CB, L], bf16, name=f"ct_{k}_{d}") for d in range(3)]
          for k in range(3)]
    wtiles = (WX, WY, WZ)
    for k in range(3):
        for d in range(3):
            if k == 2:
                ttv(ct[k][d][:], Binc[d][:], maskb[:], Alu.mult)
            else:
                ttv(ct[k][d][:], wtiles[d][:][:, k, :], Bex[d][:], Alu.add)
                ttv(ct[k][d][:], ct[k][d][:], maskb[:], Alu.mult)

    def outview(k, d):
        return out_sb[:].rearrange("p (j i q) -> p i j q", j=NB, i=G,
                                   q=9)[:, :, :, 3 * k + d]

    def pview(plane):
        return plane[:].rearrange("p (i j) -> p i j", j=NB)

    for k in range(3):
        for d in range(3):
            acopy(outview(k, d), pview(ct[k][d]))

    nc.sync.dma_start(out_ap, out_sb[:])


_CACHE = {}


def _build():
    from contextlib import ExitStack

    import concourse.bacc as bacc
    import concourse.mybir as mybir
    import concourse.tile as tile

    nc = bacc.Bacc("TRN2", target_bir_lowering=False, debug=False,
                   num_devices=N_CORES)
    inp = nc.dram_tensor("input", [CB, 3, L], mybir.dt.float32,
                         kind="ExternalInput").ap()
    lens = nc.dram_tensor("lens", [CB, 1], mybir.dt.float32,
                          kind="ExternalInput").ap()
    out = nc.dram_tensor("out", [CB, 9 * L], mybir.dt.float32,
                         kind="ExternalOutput").ap()
    with tile.TileContext(nc) as tc_ctx, ExitStack() as ctx:
        _body(ctx, tc_ctx, out, inp, lens)
    nc.compile()
    return nc


def get_nc():
    if "nc" not in _CACHE:
        _CACHE["nc"] = _build()
    return _CACHE["nc"]


def make_in_maps(input, angles_length):
    inp = np.ascontiguousarray(np.asarray(input, dtype=np.float32))
    lens = np.asarray(angles_length).astype(np.float32).reshape(B_FULL, 1)
    in_maps = []
    for i in range(N_CORES):
        sl = slice(i * CB, (i + 1) * CB)
        in_maps.append({
            "input": np.ascontiguousarray(inp[sl]),
            "lens": np.ascontiguousarray(lens[sl]),
        })
    return in_maps


def kernel(input, angles_length):
    from concourse.bass_utils import run_bass_kernel_spmd

    nc = get_nc()
    in_maps = make_in_maps(input, angles_length)
    res = run_bass_kernel_spmd(nc, in_maps, core_ids=list(range(N_CORES)))
    outs = [res.results[i]["out"] for i in range(N_CORES)]
    return np.concatenate(outs, axis=0).astype(np.float32)
